# revision 1
# baseline (speedup 1.0000x reference)
"""Trainium2 Bass kernel for nn_EnhancedDualGCN (dual 3-layer GCN, N=100k, E=1.6M).

Node-sharded across 8 NeuronCores:
  - Host renumbers nodes (balanced blocks), pads to N_PAD=784*128; core c owns
    N_LOCAL=12544 consecutive new-ids.
  - Per GCN layer: each core computes xw = h @ W^T for its shard, AllGathers the
    full [N_PAD, H] table, then processes its in-edges in 4 source-quadrant
    passes (dma_gather int16 index range), 128-edge tiles.
  - Per tile: C = gather(xw, src) [128e,128H]; S[e,r] = norm[e]*(dstwin[e]==r)
    built on VectorE; PE matmul S^T @ C -> [64 rows, H]; rows dma_scatter_add'ed
    into a per-core accumulator; then BN/relu/residual update of h.
"""

import os
import sys

sys.path.insert(0, "/opt/trn_rl_repo")

import numpy as np

H = 128
L = 3
W = 64          # scatter rows (dest windows) per tile
GG = 8          # tiles per gather/scatter group (64 descs/engine packet limit)
N_CORES = 8
BN_EPS = 1e-5
F32 = np.float32

_BRANCHES = ("call", "loc")


# ----------------------------------------------------------------------------
# Host-side preprocessing
# ----------------------------------------------------------------------------

def _branch_edges(ei, ew, n_nodes):
    src = np.concatenate([ei[0], np.arange(n_nodes, dtype=np.int64)])
    dst = np.concatenate([ei[1], np.arange(n_nodes, dtype=np.int64)])
    w = np.concatenate([ew, np.ones(n_nodes, ew.dtype)]).astype(F32)
    deg = np.zeros(n_nodes, F32)
    np.add.at(deg, dst, w)
    dis = np.where(deg > 0, 1.0 / np.sqrt(deg), 0.0).astype(F32)
    norm = (dis[src] * w * dis[dst]).astype(F32)
    return src, dst, norm


def _build_perm(deg_sum, n_pad):
    order = np.argsort(-deg_sum, kind="stable")
    n_blocks = n_pad // 128
    perm = np.empty(n_pad, dtype=np.int64)
    perm[order] = (np.arange(n_pad) % n_blocks) * 128 + np.arange(n_pad) // n_blocks
    return perm


def _pack_tiles(src_q, dstl, norm, dummy_row):
    """Greedy run packing. Returns per-tile arrays (gidx/nrm/dwin [T,128],
    sidx [T,64])."""
    o = np.lexsort((src_q, dstl))
    src_q = src_q[o].tolist()
    dstl_l = dstl[o].tolist()
    norm_l = norm[o].tolist()
    nE = len(dstl_l)

    g_rows, n_rows, w_rows, s_rows = [], [], [], []
    cg = [0] * 128
    cn = [0.0] * 128
    cw = [0] * 128
    cs = [dummy_row] * W
    fill = 0
    runs = 0

    def flush():
        nonlocal cg, cn, cw, cs, fill, runs
        g_rows.append(cg)
        n_rows.append(cn)
        w_rows.append(cw)
        s_rows.append(cs)
        cg = [0] * 128
        cn = [0.0] * 128
        cw = [0] * 128
        cs = [dummy_row] * W
        fill = 0
        runs = 0

    i = 0
    while i < nE:
        j = i + 1
        d = dstl_l[i]
        while j < nE and dstl_l[j] == d:
            j += 1
        rl = j - i
        assert rl <= 128
        if fill + rl > 128 or runs == W:
            flush()
        cg[fill:fill + rl] = src_q[i:j]
        cn[fill:fill + rl] = norm_l[i:j]
        cw[fill:fill + rl] = [runs] * rl
        cs[runs] = d
        fill += rl
        runs += 1
        i = j
    if fill or runs:
        flush()
    if not g_rows:
        flush()
    return (np.asarray(g_rows, np.int32), np.asarray(n_rows, F32),
            np.asarray(w_rows, np.int32), np.asarray(s_rows, np.int32))


def _wrap16(vals, cols):
    """[n] -> [128, n//16] int16 in the SWDGE 16-partition wrap, replicated
    across the 8 GPSIMD-core partition stripes (HW requirement)."""
    n = vals.shape[0]
    out = np.zeros((128, cols), np.int16)
    pat = vals.reshape(-1, 16).T.astype(np.int16)  # [16, n//16]
    out[:, :n // 16] = np.tile(pat, (8, 1))
    return out


def _prep_branch(ei, ew, perm, n_nodes, n_pad):
    """Returns dict q -> per-core list of (gidx,nrm,dwin,sidx [T,...]) and the
    padded uniform tile counts per q."""
    n_local = n_pad // N_CORES
    n_quad = n_pad // 4
    dummy_row = n_local  # first dummy out row
    src, dst, norm = _branch_edges(ei, ew, n_nodes)
    nsrc = perm[src]
    ndst = perm[dst]
    core = ndst // n_local
    dstl = ndst % n_local
    quad = nsrc // n_quad
    srcq = (nsrc % n_quad).astype(np.int32)

    per_q = []
    tcounts = []
    for q in range(4):
        per_core = []
        for c in range(N_CORES):
            m = (core == c) & (quad == q)
            per_core.append(_pack_tiles(srcq[m], dstl[m].astype(np.int64),
                                        norm[m], dummy_row))
        T = max(pc[0].shape[0] for pc in per_core)
        T = ((T + GG - 1) // GG) * GG
        padded = []
        for (g, n, w_, s) in per_core:
            t = g.shape[0]
            if t < T:
                g = np.concatenate([g, np.zeros((T - t, 128), np.int32)])
                n = np.concatenate([n, np.zeros((T - t, 128), F32)])
                w_ = np.concatenate([w_, np.zeros((T - t, 128), np.int32)])
                s = np.concatenate([s, np.full((T - t, W), dummy_row, np.int32)])
            padded.append((g, n, w_, s))
        per_q.append(padded)
        tcounts.append(T)
    return per_q, tcounts


def _group_arrays(g, n, w_, s):
    """Device-layout arrays for one (core, q): returns
    gidx [128, (T//GG)*128] i16, nrm [128, T] f32, dwin [128, T] f32,
    sidx [128, (T//GG)*64] i16."""
    T = g.shape[0]
    ngr = T // GG
    nrm = n.T.copy()                       # [128, T]
    dwin = w_.astype(F32).T.copy()         # [128, T]
    gc = GG * 128 // 16   # gidx cols per group
    sc = GG * W // 16     # sidx cols per group
    gidx = np.zeros((128, ngr * gc), np.int16)
    sidx = np.zeros((128, ngr * sc), np.int16)
    for gr in range(ngr):
        gv = g[gr * GG:(gr + 1) * GG].reshape(-1)  # (tile, slot) order
        gidx[:, gr * gc:(gr + 1) * gc] = _wrap16(gv, gc)
        # scatter slots: tile tg row r -> j = (h*4 + k8//2)*128 + (k8%2)*64 + r
        sv = np.empty(GG * W, np.int32)
        for tg in range(GG):
            h, k8 = tg // 8, tg % 8
            base = (h * 4 + k8 // 2) * 128 + (k8 % 2) * 64
            sv[base:base + W] = s[gr * GG + tg]
        sidx[:, gr * sc:(gr + 1) * sc] = _wrap16(sv, sc)
    return gidx, nrm, dwin, sidx


def _affine_bn(p):
    g, b, m, v = [np.asarray(x, F32) for x in (p[0], p[1], p[2], p[3])]
    A = (g / np.sqrt(v + BN_EPS)).astype(F32)
    B = (b - m * A).astype(F32)
    return A, B


def _host_prep(inputs):
    """All numpy preprocessing. Returns (meta, in_maps_common, per_core_maps,
    perm)."""
    n_nodes = inputs["x"].shape[0]
    n_pad = ((n_nodes + N_CORES * 128 - 1) // (N_CORES * 128)) * (N_CORES * 128)
    n_local = n_pad // N_CORES

    ei_c = np.asarray(inputs["edge_index_call"], np.int64)
    ei_l = np.asarray(inputs["edge_index_loc"], np.int64)
    deg_sum = np.zeros(n_pad, np.int64)
    np.add.at(deg_sum[:n_nodes], ei_c[1], 1)
    np.add.at(deg_sum[:n_nodes], ei_l[1], 1)
    perm = _build_perm(deg_sum, n_pad)

    branches = {}
    tcounts = {}
    for b, ei, ew in (("call", ei_c, inputs["edge_weight_call"]),
                      ("loc", ei_l, inputs["edge_weight_loc"])):
        per_q, tc = _prep_branch(ei, np.asarray(ew, F32), perm, n_nodes, n_pad)
        branches[b] = per_q
        tcounts[b] = tc

    # per-core device metadata arrays
    per_core_maps = [dict() for _ in range(N_CORES)]
    for b in _BRANCHES:
        for q in range(4):
            for c in range(N_CORES):
                g, n, w_, s = branches[b][q][c]
                gidx, nrm, dwin, sidx = _group_arrays(g, n, w_, s)
                pm = per_core_maps[c]
                pm[f"{b}{q}_gidx"] = gidx
                pm[f"{b}{q}_nrm"] = nrm
                pm[f"{b}{q}_dwin"] = dwin
                pm[f"{b}{q}_sidx"] = sidx

    # features (permuted, padded, transposed)
    x = np.nan_to_num(np.asarray(inputs["x"], F32))
    emb = np.asarray(inputs["emb"], F32)
    x_pad = np.zeros((n_pad, x.shape[1]), F32)
    emb_pad = np.zeros((n_pad, emb.shape[1]), F32)
    x_pad[perm[:n_nodes]] = x
    emb_pad[perm[:n_nodes]] = emb
    for c in range(N_CORES):
        sl = slice(c * n_local, (c + 1) * n_local)
        per_core_maps[c]["xT"] = np.ascontiguousarray(x_pad[sl].T)    # [16, NL]
        per_core_maps[c]["embT"] = np.ascontiguousarray(emb_pad[sl].T)  # [32, NL]

    # weights (shared across cores)
    comb_W = np.asarray(inputs["comb_W"], F32)
    Wc1 = comb_W[:, :emb.shape[1]]
    Wc2 = comb_W[:, emb.shape[1]:]
    Wx = Wc2 @ np.asarray(inputs["ft_W"], F32)      # [H, IN] (ft_b == 0)
    common = {
        "WxT": np.ascontiguousarray(Wx.T),          # [IN, H]
        "Wc1T": np.ascontiguousarray(Wc1.T),        # [EMB, H]
        "comb_b": np.asarray(inputs["comb_b"], F32).reshape(H, 1),
    }
    for b in _BRANCHES:
        Ws = np.asarray(inputs[f"{b[:4]}_W" if b == "call" else "loc_W"], F32)
        bs = np.asarray(inputs["call_b" if b == "call" else "loc_b"], F32)
        A, B = _affine_bn(np.asarray(inputs[f"bn_{b}"], F32))
        WT = np.concatenate([Ws[l].T for l in range(L)], axis=1)  # [H, L*H]
        common[f"{b}_WT"] = np.ascontiguousarray(WT)
        common[f"{b}_bias"] = np.ascontiguousarray(bs[:L].T)      # [H, L]
        common[f"{b}_A"] = np.tile(A.reshape(H, 1), (1, L))
        common[f"{b}_B"] = np.tile(B.reshape(H, 1), (1, L))
    fus_W = np.asarray(inputs["fus_W"], F32)
    Af, Bf = _affine_bn(np.asarray(inputs["bn_fus"], F32))
    common.update({
        "Wf1T": np.ascontiguousarray(fus_W[:, :H].T),
        "Wf2T": np.ascontiguousarray(fus_W[:, H:].T),
        "fus_b": np.asarray(inputs["fus_b"], F32).reshape(H, 1),
        "Af": Af.reshape(H, 1),
        "Bf": Bf.reshape(H, 1),
        "linW": np.asarray(inputs["lin_W"], F32).reshape(H, 1),
    })
    lin_b = float(np.asarray(inputs["lin_b"], F32).reshape(-1)[0])

    meta = {
        "n_nodes": n_nodes,
        "n_pad": n_pad,
        "n_local": n_local,
        "n_quad": n_pad // 4,
        "tcounts": tcounts,
        "in_dim": x.shape[1],
        "emb_dim": emb.shape[1],
        "lin_b": lin_b,
    }
    for c in range(N_CORES):
        per_core_maps[c].update(common)
    return meta, per_core_maps, perm


# ----------------------------------------------------------------------------
# Device program
# ----------------------------------------------------------------------------

def _build_program(meta):
    import concourse.bass as bass
    import concourse.bacc as bacc
    import concourse.mybir as mybir
    import concourse.tile as tile
    from concourse.masks import make_identity

    f32 = mybir.dt.float32
    bf = mybir.dt.bfloat16
    i16 = mybir.dt.int16
    i32 = mybir.dt.int32
    AOT = mybir.AluOpType
    ACT = mybir.ActivationFunctionType

    NL = meta["n_local"]
    NQ = meta["n_quad"]
    NP = meta["n_pad"]
    NB = NL // 128                   # node tiles per core
    OUT_ROWS = NL + 128
    IN_DIM = meta["in_dim"]
    EMB_DIM = meta["emb_dim"]
    tcounts = meta["tcounts"]

    nc = bacc.Bacc(None, num_devices=N_CORES)

    # ---- I/O declarations ----
    inp = {}
    def ext(name, shape, dt=f32):
        inp[name] = nc.dram_tensor(name, list(shape), dt, kind="ExternalInput")
        return inp[name]

    ext("xT", [IN_DIM, NL]); ext("embT", [EMB_DIM, NL])
    ext("WxT", [IN_DIM, H]); ext("Wc1T", [EMB_DIM, H]); ext("comb_b", [H, 1])
    for b in _BRANCHES:
        ext(f"{b}_WT", [H, L * H]); ext(f"{b}_bias", [H, L])
        ext(f"{b}_A", [H, L]); ext(f"{b}_B", [H, L])
        for q in range(4):
            T = tcounts[b][q]
            ext(f"{b}{q}_gidx", [128, (T // GG) * (GG * 128 // 16)], i16)
            ext(f"{b}{q}_nrm", [128, T])
            ext(f"{b}{q}_dwin", [128, T])
            ext(f"{b}{q}_sidx", [128, (T // GG) * (GG * W // 16)], i16)
    ext("Wf1T", [H, H]); ext("Wf2T", [H, H]); ext("fus_b", [H, 1])
    ext("Af", [H, 1]); ext("Bf", [H, 1]); ext("linW", [H, 1])
    y_out = nc.dram_tensor("y", [1, NL], f32, kind="ExternalOutput")

    # internal DRAM
    xw_bounce = [nc.dram_tensor(f"xw_bounce{i}", [NL, H], f32) for i in range(2)]
    xw_full = [nc.dram_tensor(f"xw_full{i}", [NP, H], f32, addr_space="Shared")
               for i in range(2)]
    out_acc = [[nc.dram_tensor(f"out_acc{i}_{p}", [OUT_ROWS, H], f32)
                for p in range(2)] for i in range(2)]

    steps = [(b, l) for l in range(L) for b in _BRANCHES]

    with tile.TileContext(nc) as tc:
        import contextlib
        with contextlib.ExitStack() as ctx:
            cpool = ctx.enter_context(tc.tile_pool(name="cpool", bufs=2))
            spool = ctx.enter_context(tc.tile_pool(name="spool", bufs=2))
            stgpool = ctx.enter_context(tc.tile_pool(name="stgpool", bufs=2))
            mpool = ctx.enter_context(tc.tile_pool(name="mpool", bufs=2))
            upool = ctx.enter_context(tc.tile_pool(name="upool", bufs=2))
            xpool = ctx.enter_context(tc.tile_pool(name="xpool", bufs=2))
            konst = ctx.enter_context(tc.tile_pool(name="konst", bufs=1))
            hpool = ctx.enter_context(tc.tile_pool(name="hpool", bufs=1))
            pspool = ctx.enter_context(tc.tile_pool(name="ps", bufs=6, space="PSUM"))

            # ---- constants ----
            ident = konst.tile([128, 128], f32, tag="ident", name="ident")
            make_identity(nc, ident[:])
            iota_i = konst.tile([128, W], i32, tag="iota_i", name="iota_i")
            nc.gpsimd.iota(iota_i[:], pattern=[[1, W]], base=0, channel_multiplier=0)
            iota_f = konst.tile([128, W], f32, tag="iota_f", name="iota_f")
            nc.vector.tensor_copy(out=iota_f[:], in_=iota_i[:])
            zeros = konst.tile([128, 3 * 128], f32, tag="zeros", name="zeros")
            nc.vector.memset(zeros[:], 0.0)

            WT_sb = {}
            bias_sb = {}
            A_sb = {}
            B_sb = {}
            for b in _BRANCHES:
                WT_sb[b] = konst.tile([H, L * H], f32, tag=f"WT_{b}", name=f"WT_{b}")
                nc.sync.dma_start(out=WT_sb[b][:], in_=inp[f"{b}_WT"][:, :])
                bias_sb[b] = konst.tile([H, L], f32, tag=f"bias_{b}", name=f"bias_{b}")
                nc.sync.dma_start(out=bias_sb[b][:], in_=inp[f"{b}_bias"][:, :])
                A_sb[b] = konst.tile([H, L], f32, tag=f"A_{b}", name=f"A_{b}")
                nc.sync.dma_start(out=A_sb[b][:], in_=inp[f"{b}_A"][:, :])
                B_sb[b] = konst.tile([H, L], f32, tag=f"B_{b}", name=f"B_{b}")
                nc.sync.dma_start(out=B_sb[b][:], in_=inp[f"{b}_B"][:, :])
            Wf1T_sb = konst.tile([H, H], f32, tag="wf1", name="wf1")
            Wf2T_sb = konst.tile([H, H], f32, tag="wf2", name="wf2")
            nc.sync.dma_start(out=Wf1T_sb[:], in_=inp["Wf1T"][:, :])
            nc.sync.dma_start(out=Wf2T_sb[:], in_=inp["Wf2T"][:, :])
            fus_b_sb = konst.tile([H, 1], f32, tag="fusb", name="fusb")
            nc.sync.dma_start(out=fus_b_sb[:], in_=inp["fus_b"][:, :])
            Af_sb = konst.tile([H, 1], f32, tag="af", name="af")
            nc.sync.dma_start(out=Af_sb[:], in_=inp["Af"][:, :])
            Bf_sb = konst.tile([H, 1], f32, tag="bf", name="bf")
            nc.sync.dma_start(out=Bf_sb[:], in_=inp["Bf"][:, :])
            linW_sb = konst.tile([H, 1], f32, tag="linw", name="linw")
            nc.sync.dma_start(out=linW_sb[:], in_=inp["linW"][:, :])
            comb_b_sb = konst.tile([H, 1], f32, tag="combb", name="combb")
            nc.sync.dma_start(out=comb_b_sb[:], in_=inp["comb_b"][:, :])

            hT = {b: hpool.tile([128, NL], f32, tag=f"hT_{b}", name=f"hT_{b}") for b in _BRANCHES}

            reg_g = nc.gpsimd.to_reg(GG * 128)
            reg_s = nc.gpsimd.to_reg(GG * W)
            qrr = [0]

            # ---- front: h0 = relu(emb@Wc1^T + x@Wx^T + comb_b), feature-major ----
            with tc.tile_pool(name="front", bufs=2) as fpool:
                WxT_sb = fpool.tile([IN_DIM, H], f32, tag="WxT", name="WxT",
                                    bufs=1)
                Wc1T_sb = fpool.tile([EMB_DIM, H], f32, tag="Wc1T", name="Wc1T",
                                     bufs=1)
                nc.sync.dma_start(out=WxT_sb[:], in_=inp["WxT"][:, :])
                nc.sync.dma_start(out=Wc1T_sb[:], in_=inp["Wc1T"][:, :])
                for c0 in range(0, NL, 512):
                    cw = min(512, NL - c0)
                    xT_sb = fpool.tile([IN_DIM, 512], f32, tag="xT", name="xT")
                    embT_sb = fpool.tile([EMB_DIM, 512], f32, tag="embT",
                                         name="embT")
                    nc.sync.dma_start(out=xT_sb[:, :cw],
                                      in_=inp["xT"][:, c0:c0 + cw])
                    nc.sync.dma_start(out=embT_sb[:, :cw],
                                      in_=inp["embT"][:, c0:c0 + cw])
                    ps = pspool.tile([128, 512], f32, tag="ps", name="ps")
                    nc.tensor.matmul(out=ps[:, :cw], lhsT=WxT_sb[:],
                                     rhs=xT_sb[:, :cw],
                                     start=True, stop=False)
                    nc.tensor.matmul(out=ps[:, :cw], lhsT=Wc1T_sb[:],
                                     rhs=embT_sb[:, :cw],
                                     start=False, stop=True)
                    nc.scalar.activation(out=hT["call"][:, c0:c0 + cw],
                                         in_=ps[:, :cw], func=ACT.Relu,
                                         bias=comb_b_sb[:, 0:1], scale=1.0)
                    nc.vector.tensor_copy(out=hT["loc"][:, c0:c0 + cw],
                                          in_=hT["call"][:, c0:c0 + cw])

            # ---- main loop ----
            def phase_a(k):
                b, l = steps[k]
                pp = k % 2
                # zero out_acc[pp] (11 chunks of 9 node-blocks)
                for oa in out_acc[pp]:
                    for r0 in range(0, OUT_ROWS, 3 * 128):
                        rw = min(3 * 128, OUT_ROWS - r0)
                        nb = rw // 128
                        dst = oa[r0:r0 + rw, :]
                        nc.sync.dma_start(
                            out=dst.rearrange("(a p) h -> p a h", p=128),
                            in_=zeros[:].rearrange("p (a h) -> p a h", h=128)[:, :nb, :])
                # xw = h @ W_l^T  (node-major), batched 4 node tiles
                for c0 in range(0, NL, 512):
                    cw = min(512, NL - c0)
                    nt = cw // 128
                    ps = pspool.tile([128, 512], f32, tag="ps", name="ps")
                    for t in range(nt):
                        nc.tensor.matmul(
                            out=ps[:, t * 128:(t + 1) * 128],
                            lhsT=hT[b][:, c0 + t * 128:c0 + (t + 1) * 128],
                            rhs=WT_sb[b][:, l * H:(l + 1) * H],
                            start=True, stop=True)
                    stg = xpool.tile([128, 4, 128], f32, tag="xwstg", name="xwstg")
                    nc.vector.tensor_copy(out=stg[:, :nt, :], in_=ps[:, :cw])
                    nc.sync.dma_start(
                        out=xw_bounce[pp][c0:c0 + cw, :].rearrange(
                            "(t p) h -> p t h", p=128),
                        in_=stg[:, :nt, :])
                nc.gpsimd.collective_compute(
                    "AllGather", AOT.bypass,
                    replica_groups=[list(range(N_CORES))],
                    ins=[xw_bounce[pp][:, :].opt()],
                    outs=[xw_full[pp][:, :].opt()],
                )

            def phase_b(k):
                b, l = steps[k]
                pp = k % 2
                gcnt = 0
                for q in range(4):
                    T = tcounts[b][q]
                    ngr = T // GG
                    table = xw_full[pp][q * NQ:(q + 1) * NQ, :]
                    # whole-pass metadata
                    gcc = GG * 128 // 16
                    scc = GG * W // 16
                    gidx_sb = mpool.tile([128, ngr * gcc], i16, tag="gidx", name="gidx", bufs=1)
                    nrm_sb = mpool.tile([128, T], f32, tag="nrm", name="nrm")
                    dwin_sb = mpool.tile([128, T], f32, tag="dwin", name="dwin")
                    sidx_sb = mpool.tile([128, ngr * scc], i16, tag="sidx", name="sidx")
                    nc.sync.dma_start(out=gidx_sb[:], in_=inp[f"{b}{q}_gidx"][:, :])
                    nc.sync.dma_start(out=nrm_sb[:], in_=inp[f"{b}{q}_nrm"][:, :])
                    nc.sync.dma_start(out=dwin_sb[:], in_=inp[f"{b}{q}_dwin"][:, :])
                    nc.sync.dma_start(out=sidx_sb[:], in_=inp[f"{b}{q}_sidx"][:, :])
                    for gr in range(ngr):
                        C = cpool.tile([128, GG, 128], f32, tag="C", name="C", bufs=4)
                        nc.gpsimd.dma_gather(
                            C[:], table,
                            gidx_sb[:, gr * gcc:(gr + 1) * gcc],
                            GG * 128, reg_g, H)
                        S = spool.tile([128, GG, W], f32, tag="S", name="S", bufs=4)
                        t0 = gr * GG
                        nc.vector.tensor_tensor(
                            out=S[:],
                            in0=iota_f[:, None, :].to_broadcast([128, GG, W]),
                            in1=dwin_sb[:, t0:t0 + GG, None].to_broadcast(
                                [128, GG, W]),
                            op=AOT.is_equal)
                        nc.vector.tensor_tensor(
                            out=S[:], in0=S[:],
                            in1=nrm_sb[:, t0:t0 + GG, None].to_broadcast(
                                [128, GG, W]),
                            op=AOT.mult)
                        stg = stgpool.tile([128, GG // 2, 128], f32, tag="stg", name="stg", bufs=4)
                        for half in range(GG // 8):
                            ps = pspool.tile([128, 512], f32, tag="ps", name="ps")
                            for k8 in range(8):
                                t = half * 8 + k8
                                po = (k8 % 2) * 64
                                fo = (k8 // 2) * 128
                                nc.tensor.matmul(
                                    out=ps[po:po + 64, fo:fo + 128],
                                    lhsT=S[:, t, :], rhs=C[:, t, :],
                                    start=True, stop=True)
                            nc.vector.tensor_copy(
                                out=stg[:, half * 4:(half + 1) * 4, :],
                                in_=ps[:, :])
                        nc.gpsimd.dma_scatter_add(
                            out_acc[pp][gcnt % 2][:, :], stg[:],
                            sidx_sb[:, gr * scc:(gr + 1) * scc],
                            GG * W, reg_s, H)
                        gcnt += 1
                # update: h += BN(relu(acc + bias))
                for c0 in range(0, NL, 512):
                    cw = min(512, NL - c0)
                    nt = cw // 128
                    ps = pspool.tile([128, 512], f32, tag="ps", name="ps")
                    osbs = []
                    for p_ in range(2):
                        osb = upool.tile([128, 4, 128], f32, tag=f"osb{p_}",
                                         name="osb", bufs=2)
                        nc.sync.dma_start(
                            out=osb[:, :nt, :],
                            in_=out_acc[pp][p_][c0:c0 + cw, :].rearrange(
                                "(t p) h -> p t h", p=128))
                        osbs.append(osb)
                    for t in range(nt):
                        for p_ in range(2):
                            nc.tensor.matmul(
                                out=ps[:, t * 128:(t + 1) * 128],
                                lhsT=osbs[p_][:, t, :], rhs=ident[:],
                                is_transpose=True,
                                start=(p_ == 0), stop=(p_ == 1))
                    tmp = upool.tile([128, 512], f32, tag="utmp", name="utmp")
                    nc.scalar.activation(out=tmp[:, :cw], in_=ps[:, :cw],
                                         func=ACT.Relu,
                                         bias=bias_sb[b][:, l:l + 1], scale=1.0)
                    nc.vector.tensor_scalar(
                        out=tmp[:, :cw], in0=tmp[:, :cw],
                        scalar1=A_sb[b][:, l:l + 1],
                        scalar2=B_sb[b][:, l:l + 1],
                        op0=AOT.mult, op1=AOT.add)
                    nc.vector.tensor_tensor(
                        out=hT[b][:, c0:c0 + cw], in0=hT[b][:, c0:c0 + cw],
                        in1=tmp[:, :cw], op=AOT.add)

            phase_a(0)
            phase_a(1)
            for k in range(2, len(steps)):
                phase_b(k - 2)
                phase_a(k)
            phase_b(len(steps) - 2)
            phase_b(len(steps) - 1)

            # ---- back: fuse + BN + head ----
            lin_b = meta["lin_b"]
            for c0 in range(0, NL, 512):
                cw = min(512, NL - c0)
                ps = pspool.tile([128, 512], f32, tag="ps", name="ps")
                nc.tensor.matmul(out=ps[:, :cw], lhsT=Wf1T_sb[:],
                                 rhs=hT["call"][:, c0:c0 + cw],
                                 start=True, stop=False)
                nc.tensor.matmul(out=ps[:, :cw], lhsT=Wf2T_sb[:],
                                 rhs=hT["loc"][:, c0:c0 + cw],
                                 start=False, stop=True)
                hf = upool.tile([128, 512], f32, tag="hf", name="hf")
                nc.scalar.activation(out=hf[:, :cw], in_=ps[:, :cw],
                                     func=ACT.Relu, bias=fus_b_sb[:, 0:1],
                                     scale=1.0)
                nc.vector.tensor_scalar(out=hf[:, :cw], in0=hf[:, :cw],
                                        scalar1=Af_sb[:, 0:1],
                                        scalar2=Bf_sb[:, 0:1],
                                        op0=AOT.mult, op1=AOT.add)
                psy = pspool.tile([1, 512], f32, tag="ps", name="psy")
                nc.tensor.matmul(out=psy[:, :cw], lhsT=linW_sb[:],
                                 rhs=hf[:, :cw], start=True, stop=True)
                ytmp = upool.tile([1, 512], f32, tag="ytmp", name="ytmp")
                nc.scalar.activation(out=ytmp[:, :cw], in_=psy[:, :cw],
                                     func=ACT.Copy, bias=lin_b, scale=1.0)
                ycl = upool.tile([1, 512], f32, tag="ycl", name="ycl")
                nc.vector.tensor_scalar(out=ycl[:, :cw],
                                        in0=ytmp[:, :cw],
                                        scalar1=-10.0, scalar2=10.0,
                                        op0=AOT.max, op1=AOT.min)
                nc.sync.dma_start(out=y_out[:, c0:c0 + cw], in_=ycl[:, :cw])
            if os.environ.get("GCN_DEBUG_H") == "1":
                dbg_c = nc.dram_tensor("dbg_hcall", [128, NL], f32,
                                       kind="ExternalOutput")
                dbg_l = nc.dram_tensor("dbg_hloc", [128, NL], f32,
                                       kind="ExternalOutput")
                nc.sync.dma_start(out=dbg_c[:, :], in_=hT["call"][:])
                nc.sync.dma_start(out=dbg_l[:, :], in_=hT["loc"][:])

    nc.compile()
    return nc


# ----------------------------------------------------------------------------
# Entry point
# ----------------------------------------------------------------------------

def kernel(**inputs) -> np.ndarray:
    from concourse.bass_utils import run_bass_kernel_spmd

    meta, per_core_maps, perm = _host_prep(inputs)
    nc = _build_program(meta)
    trace = os.environ.get("GCN_TRACE", "") == "1"
    kw = {}
    if trace:
        kw = dict(trace=True)
    res = run_bass_kernel_spmd(nc, per_core_maps,
                               core_ids=list(range(N_CORES)), **kw)
    if trace:
        kernel.last_exec_time_ns = res.exec_time_ns
        kernel.last_trace = (res.instructions_and_trace[1]
                             if res.instructions_and_trace else None)
    n_nodes = meta["n_nodes"]
    y_pad = np.concatenate([res.results[c]["y"].reshape(-1)
                            for c in range(N_CORES)])
    out = y_pad[perm[:n_nodes]].astype(np.float32).reshape(n_nodes, 1)
    return out



# revision 12
# speedup vs baseline: 1.9682x; 1.9682x over previous
"""Trainium2 Bass kernel for nn_EnhancedDualGCN (dual 3-layer GCN, N=100k, E=1.6M).

Node-sharded across 8 NeuronCores, v2 (scatter-free):
  - Host renumbers nodes (balanced blocks), pads to N_PAD=784*128; core c owns
    NL=12544 consecutive new-ids. h kept NODE-major in SBUF ([128, 98, 128]).
  - Per GCN layer: xw = h @ W^T per shard (PE transpose of h blocks + matmul),
    AllGather the full [N_PAD, H] bf16 table.
  - Edges sorted by (dst-superblock, src-quadrant, dst-block); 128-edge tiles.
    Per tile: C = dma_gather(xw_full_quadrant, src) [128e, H] (4 SWDGE queues);
    S[e, d] = nrm[e] * (dwin[e] == d) built by one DVE tensor_scalar;
    PE matmul lhsT=S rhs=C accumulates node-major [dst, H] into the
    superblock's PSUM tile. No dma_scatter: PSUM accumulation replaces it.
  - Update: h += BN(relu(psum + bias)) with feature-wise BN applied through
    host-replicated [128,128] constant tiles (node-major friendly).
"""

import os
import sys

sys.path.insert(0, "/opt/trn_rl_repo")

import numpy as np

H = 128
L = 3
N_CORES = 8
BN_EPS = 1e-5
F32 = np.float32
BF16 = None  # set lazily (ml_dtypes)

_BRANCHES = ("call", "loc")


# ----------------------------------------------------------------------------
# Host-side preprocessing
# ----------------------------------------------------------------------------

def _branch_edges(ei, ew, n_nodes):
    src = np.concatenate([ei[0], np.arange(n_nodes, dtype=np.int64)])
    dst = np.concatenate([ei[1], np.arange(n_nodes, dtype=np.int64)])
    w = np.concatenate([ew, np.ones(n_nodes, ew.dtype)]).astype(F32)
    deg = np.zeros(n_nodes, F32)
    np.add.at(deg, dst, w)
    dis = np.where(deg > 0, 1.0 / np.sqrt(deg), 0.0).astype(F32)
    norm = (dis[src] * w * dis[dst]).astype(F32)
    return src, dst, norm


def _build_perm(deg_sum, n_pad):
    order = np.argsort(-deg_sum, kind="stable")
    n_blocks = n_pad // 128
    perm = np.empty(n_pad, dtype=np.int64)
    perm[order] = (np.arange(n_pad) % n_blocks) * 128 + np.arange(n_pad) // n_blocks
    return perm


def _wrap16(vals):
    """[n] int -> [128, n//16] int16: 16-partition wrap, replicated across the
    8 GPSIMD core stripes (HW requirement)."""
    pat = vals.reshape(-1, 16).T.astype(np.int16)
    return np.tile(pat, (8, 1))


def _prep_branch(ei, ew, perm, n_nodes, n_pad):
    """Edge metadata for one branch.

    Returns (T, per_core) where T[q][blk] = uniform (max-over-core) tile count
    and per_core[c] = dict(gidx [128, TT*8] i16, nrm [128, TT] bf16,
    dwin [128, TT] bf16) laid out in (sb, q, blk, tile) order.
    """
    NL = n_pad // N_CORES
    NQ = n_pad // 4
    NBLK = NL // 128
    src, dst, norm = _branch_edges(ei, ew, n_nodes)
    nsrc = perm[src]
    ndst = perm[dst]
    core = ndst // NL
    dstl = ndst % NL
    q = nsrc // NQ
    srcq = (nsrc % NQ).astype(np.int32)
    blk = dstl // 128

    # counts per (core, q, blk)
    counts = np.zeros((N_CORES, 4, NBLK), np.int64)
    np.add.at(counts, (core, q, blk), 1)
    tiles = -(-counts // 128)  # ceil
    T = tiles.max(axis=0)  # [4, NBLK] uniform tile counts

    n_sb = (NBLK + 3) // 4
    # processing order: sb, q, blk
    order_blocks = []
    for sb in range(n_sb):
        for qq in range(4):
            for b in range(sb * 4, min(sb * 4 + 4, NBLK)):
                order_blocks.append((qq, b))
    TT = int(sum(T[qq][b] for qq, b in order_blocks))

    per_core = []
    for c in range(N_CORES):
        m = core == c
        # sort this core's edges by (q, blk, dstl) -> group per (q, blk)
        o = np.lexsort((dstl[m], blk[m], q[m]))
        cs = srcq[m][o]
        cn = norm[m][o]
        cd = (dstl[m][o] % 128).astype(F32)
        cq = q[m][o]
        cb = blk[m][o]
        # start offset of each (q, blk) group
        gidx = np.zeros(TT * 128, np.int32)
        nrm_a = np.zeros(TT * 128, F32)
        dwin_a = np.zeros(TT * 128, F32)
        bounds = np.searchsorted(cq * NBLK + cb, np.arange(4 * NBLK + 1),
                                 side="left",
                                 sorter=None)
        pos = 0
        for qq, b in order_blocks:
            lo, hi = bounds[qq * NBLK + b], bounds[qq * NBLK + b + 1]
            cnt = hi - lo
            t_need = T[qq][b]
            gidx[pos:pos + cnt] = cs[lo:hi]
            nrm_a[pos:pos + cnt] = cn[lo:hi]
            dwin_a[pos:pos + cnt] = cd[lo:hi]
            pos += t_need * 128
        assert pos == TT * 128
        per_core.append({
            "gidx": _wrap16(gidx),
            "nrm": np.ascontiguousarray(
                nrm_a.reshape(TT, 128).T).astype(np.float32),
            "dwin": np.ascontiguousarray(
                dwin_a.reshape(TT, 128).T).astype(np.float32),
        })
    return T, TT, per_core


def _affine_bn(p):
    g, b, m, v = [np.asarray(x, F32) for x in (p[0], p[1], p[2], p[3])]
    A = (g / np.sqrt(v + BN_EPS)).astype(F32)
    B = (b - m * A).astype(F32)
    return A, B


def _rep(v):
    """[H] -> [128, H] replicated f32 const tile."""
    return np.ascontiguousarray(np.tile(np.asarray(v, F32).reshape(1, H),
                                        (128, 1)))


def _host_prep(inputs):
    n_nodes = inputs["x"].shape[0]
    n_pad = ((n_nodes + N_CORES * 128 - 1) // (N_CORES * 128)) * (N_CORES * 128)
    NL = n_pad // N_CORES

    ei_c = np.asarray(inputs["edge_index_call"], np.int64)
    ei_l = np.asarray(inputs["edge_index_loc"], np.int64)
    deg_sum = np.zeros(n_pad, np.int64)
    np.add.at(deg_sum[:n_nodes], ei_c[1], 1)
    np.add.at(deg_sum[:n_nodes], ei_l[1], 1)
    perm = _build_perm(deg_sum, n_pad)

    T = {}
    TT = {}
    per_core_maps = [dict() for _ in range(N_CORES)]
    for b, ei, ew in (("call", ei_c, inputs["edge_weight_call"]),
                      ("loc", ei_l, inputs["edge_weight_loc"])):
        Tb, TTb, pc = _prep_branch(ei, np.asarray(ew, F32), perm, n_nodes,
                                   n_pad)
        T[b] = Tb
        TT[b] = TTb
        for c in range(N_CORES):
            per_core_maps[c][f"{b}_gidx"] = pc[c]["gidx"]
            per_core_maps[c][f"{b}_nrm"] = pc[c]["nrm"]
            per_core_maps[c][f"{b}_dwin"] = pc[c]["dwin"]

    # features (permuted, padded, feature-major)
    x = np.nan_to_num(np.asarray(inputs["x"], F32))
    emb = np.asarray(inputs["emb"], F32)
    x_pad = np.zeros((n_pad, x.shape[1]), F32)
    emb_pad = np.zeros((n_pad, emb.shape[1]), F32)
    x_pad[perm[:n_nodes]] = x
    emb_pad[perm[:n_nodes]] = emb
    for c in range(N_CORES):
        sl = slice(c * NL, (c + 1) * NL)
        per_core_maps[c]["xT"] = np.ascontiguousarray(x_pad[sl].T)
        per_core_maps[c]["embT"] = np.ascontiguousarray(emb_pad[sl].T)

    # weights (shared)
    comb_W = np.asarray(inputs["comb_W"], F32)
    Wc1 = comb_W[:, :emb.shape[1]]
    Wc2 = comb_W[:, emb.shape[1]:]
    Wx = Wc2 @ np.asarray(inputs["ft_W"], F32)  # [H, IN] (ft_b == 0)
    common = {
        "WxT": np.ascontiguousarray(Wx.T),
        "Wc1T": np.ascontiguousarray(Wc1.T),
        "comb_b_rep": _rep(inputs["comb_b"]),
    }
    for b in _BRANCHES:
        Ws = np.asarray(inputs["call_W" if b == "call" else "loc_W"], F32)
        bs = np.asarray(inputs["call_b" if b == "call" else "loc_b"], F32)
        A, B = _affine_bn(np.asarray(inputs[f"bn_{b}"], F32))
        WT = np.concatenate([Ws[l].T for l in range(L)], axis=1)  # [H, L*H]
        common[f"{b}_WT"] = np.ascontiguousarray(WT)
        for l in range(L):
            common[f"{b}{l}_bias_rep"] = _rep(bs[l])
        common[f"{b}_A_rep"] = _rep(A)
        common[f"{b}_B_rep"] = _rep(B)
    fus_W = np.asarray(inputs["fus_W"], F32)
    Af, Bf = _affine_bn(np.asarray(inputs["bn_fus"], F32))
    common.update({
        "Wf1T": np.ascontiguousarray(fus_W[:, :H].T),
        "Wf2T": np.ascontiguousarray(fus_W[:, H:].T),
        "fus_b_rep": _rep(inputs["fus_b"]),
        "Af_rep": _rep(Af),
        "Bf_rep": _rep(Bf),
        "linW_rep": _rep(np.asarray(inputs["lin_W"], F32).reshape(H)),
    })
    lin_b = float(np.asarray(inputs["lin_b"], F32).reshape(-1)[0])

    meta = {
        "n_nodes": n_nodes,
        "n_pad": n_pad,
        "NL": NL,
        "NQ": n_pad // 4,
        "NBLK": NL // 128,
        "T": T,
        "TT": TT,
        "in_dim": x.shape[1],
        "emb_dim": emb.shape[1],
        "lin_b": lin_b,
    }
    for c in range(N_CORES):
        per_core_maps[c].update(common)
    return meta, per_core_maps, perm


# ----------------------------------------------------------------------------
# Device program
# ----------------------------------------------------------------------------

def _build_program(meta):
    import contextlib

    import concourse.bass as bass
    import concourse.bacc as bacc
    import concourse.mybir as mybir
    import concourse.tile as tile
    from concourse.masks import make_identity

    f32 = mybir.dt.float32
    bf = mybir.dt.bfloat16
    i16 = mybir.dt.int16
    AOT = mybir.AluOpType
    ACT = mybir.ActivationFunctionType

    NL = meta["NL"]
    NQ = meta["NQ"]
    NP = meta["n_pad"]
    NBLK = meta["NBLK"]
    NSB = (NBLK + 3) // 4
    IN_DIM = meta["in_dim"]
    EMB_DIM = meta["emb_dim"]
    T = meta["T"]
    TT = meta["TT"]

    nc = bacc.Bacc(None, num_devices=N_CORES, num_swdge_queues=4)

    inp = {}

    def ext(name, shape, dt=f32):
        inp[name] = nc.dram_tensor(name, list(shape), dt, kind="ExternalInput")
        return inp[name]

    ext("xT", [IN_DIM, NL])
    ext("embT", [EMB_DIM, NL])
    ext("WxT", [IN_DIM, H])
    ext("Wc1T", [EMB_DIM, H])
    ext("comb_b_rep", [128, H])
    for b in _BRANCHES:
        ext(f"{b}_WT", [H, L * H])
        for l in range(L):
            ext(f"{b}{l}_bias_rep", [128, H])
        ext(f"{b}_A_rep", [128, H])
        ext(f"{b}_B_rep", [128, H])
        ext(f"{b}_gidx", [128, TT[b] * 8], i16)
        ext(f"{b}_nrm", [128, TT[b]])
        ext(f"{b}_dwin", [128, TT[b]])
    ext("Wf1T", [H, H])
    ext("Wf2T", [H, H])
    ext("fus_b_rep", [128, H])
    ext("Af_rep", [128, H])
    ext("Bf_rep", [128, H])
    ext("linW_rep", [128, H])
    y_out = nc.dram_tensor("y", [128, NBLK], f32, kind="ExternalOutput")

    xw_bounce = [nc.dram_tensor(f"xw_bounce{i}", [NL, H], bf)
                 for i in range(2)]
    xw_full = [nc.dram_tensor(f"xw_full{i}", [NP, H], bf, addr_space="Shared")
               for i in range(2)]

    steps = [(b, l) for l in range(L) for b in _BRANCHES]
    steps = steps[:int(os.environ.get("GCN_STEPS", str(len(steps))))]

    # per-branch codegen schedule: list of (sb, q, blk_list, tile_gt_list)
    # and per (sb, bi): total tiles (for start/stop flags)
    sched = {}
    for b in _BRANCHES:
        Tb = T[b]
        calls = []
        sbbi_total = np.zeros((NSB, 4), np.int64)
        gt = 0
        for sb in range(NSB):
            blks = list(range(sb * 4, min(sb * 4 + 4, NBLK)))
            for q in range(4):
                tl = []
                for blk in blks:
                    for _ in range(int(Tb[q][blk])):
                        tl.append((blk, gt))
                        gt += 1
                        sbbi_total[sb][blk - sb * 4] += 1
                calls.append((sb, q, tl))
        assert gt == TT[b]
        sched[b] = (calls, sbbi_total)

    with tile.TileContext(nc) as tc:
        with contextlib.ExitStack() as ctx:
            konst = ctx.enter_context(tc.tile_pool(name="konst", bufs=1))
            hpool = ctx.enter_context(tc.tile_pool(name="hpool", bufs=1))
            mpool = ctx.enter_context(tc.tile_pool(name="mpool", bufs=1))
            cpool = ctx.enter_context(tc.tile_pool(name="cpool", bufs=2))
            spool = ctx.enter_context(tc.tile_pool(name="spool", bufs=8))
            upool = ctx.enter_context(tc.tile_pool(name="upool", bufs=2))
            xpool = ctx.enter_context(tc.tile_pool(name="xpool", bufs=2))
            pspool = ctx.enter_context(
                tc.tile_pool(name="ps", bufs=3, space="PSUM"))
            psa = ctx.enter_context(
                tc.tile_pool(name="psa", bufs=2, space="PSUM"))

            # ---- constants ----
            ident = konst.tile([128, 128], f32, tag="ident", name="ident")
            make_identity(nc, ident[:])
            iota_i = konst.tile([128, 128], mybir.dt.int32, tag="iota_i",
                                name="iota_i")
            nc.gpsimd.iota(iota_i[:], pattern=[[1, 128]], base=0,
                           channel_multiplier=0)
            iota_b = konst.tile([128, 128], bf, tag="iota_b", name="iota_b")
            nc.vector.tensor_copy(out=iota_b[:], in_=iota_i[:])

            def kload(name, shape, dt=f32):
                t = konst.tile(shape, dt, tag=name, name=name)
                nc.sync.dma_start(out=t[:], in_=inp[name][:, :])
                return t

            WT_sb = {b: kload(f"{b}_WT", [H, L * H]) for b in _BRANCHES}
            bias_rep = {(b, l): kload(f"{b}{l}_bias_rep", [128, H])
                        for b in _BRANCHES for l in range(L)}
            A_rep = {b: kload(f"{b}_A_rep", [128, H]) for b in _BRANCHES}
            B_rep = {b: kload(f"{b}_B_rep", [128, H]) for b in _BRANCHES}
            WxT_sb = kload("WxT", [IN_DIM, H])
            Wc1T_sb = kload("Wc1T", [EMB_DIM, H])
            comb_b_rep = kload("comb_b_rep", [128, H])
            Wf1T_sb = kload("Wf1T", [H, H])
            Wf2T_sb = kload("Wf2T", [H, H])
            fus_b_rep = kload("fus_b_rep", [128, H])
            Af_rep = kload("Af_rep", [128, H])
            Bf_rep = kload("Bf_rep", [128, H])
            linW_rep = kload("linW_rep", [128, H])

            TTmax = max(TT[b] for b in _BRANCHES)

            h_sb = {b: hpool.tile([128, NBLK, 128], f32, tag=f"h_{b}",
                                  name=f"h_{b}") for b in _BRANCHES}

            # ---- front: h0 = relu(emb@Wc1^T + x@Wx^T + comb_b), node-major --
            with tc.tile_pool(name="front", bufs=1) as fpool:
                for c0 in range(0, NBLK, 4):
                    nt = min(4, NBLK - c0)
                    n0 = c0 * 128
                    cw = nt * 128
                    xT_sb = fpool.tile([IN_DIM, 4 * 128], f32, tag="xT",
                                       name="xT")
                    embT_sb = fpool.tile([EMB_DIM, 4 * 128], f32, tag="embT",
                                         name="embT")
                    nc.sync.dma_start(out=xT_sb[:, :cw],
                                      in_=inp["xT"][:, n0:n0 + cw])
                    nc.sync.dma_start(out=embT_sb[:, :cw],
                                      in_=inp["embT"][:, n0:n0 + cw])
                    ps = psa.tile([128, 4, 128], f32, tag="pst", name="pst")
                    for t in range(nt):
                        nc.tensor.matmul(out=ps[:, t, :],
                                         lhsT=xT_sb[:, t * 128:(t + 1) * 128],
                                         rhs=WxT_sb[:],
                                         start=True, stop=False)
                        nc.tensor.matmul(out=ps[:, t, :],
                                         lhsT=embT_sb[:, t * 128:(t + 1) * 128],
                                         rhs=Wc1T_sb[:],
                                         start=False, stop=True)
                    tmp = upool.tile([128, 4, 128], f32, tag="ft",
                                     name="ft")
                    nc.vector.tensor_tensor(
                        out=tmp[:, :nt, :], in0=ps[:, :nt, :],
                        in1=comb_b_rep[:, None, :].to_broadcast(
                            [128, nt, 128]),
                        op=AOT.add)
                    nc.vector.tensor_scalar(
                        out=h_sb["call"][:, c0:c0 + nt, :], in0=tmp[:, :nt, :],
                        scalar1=0.0, scalar2=None, op0=AOT.max)
                    nc.vector.tensor_copy(out=h_sb["loc"][:, c0:c0 + nt, :],
                                          in_=h_sb["call"][:, c0:c0 + nt, :])

            # ---- phases ----
            def phase_a(k):
                b, l = steps[k]
                pp = k % 2
                for c0 in range(0, NBLK, 4):
                    nt = min(4, NBLK - c0)
                    cw = nt * 128
                    pst = psa.tile([128, 4, 128], f32, tag="pst", name="pst")
                    for t in range(nt):
                        nc.tensor.matmul(out=pst[:, t, :],
                                         lhsT=h_sb[b][:, c0 + t, :],
                                         rhs=ident[:], is_transpose=True,
                                         start=True, stop=True)
                    hT = xpool.tile([128, 4, 128], f32, tag="hT", name="hT")
                    nc.scalar.activation(out=hT[:, :nt, :], in_=pst[:, :nt, :],
                                         func=ACT.Copy, bias=0.0, scale=1.0)
                    ps2 = psa.tile([128, 4, 128], f32, tag="ps2", name="ps2")
                    for t in range(nt):
                        nc.tensor.matmul(out=ps2[:, t, :],
                                         lhsT=hT[:, t, :],
                                         rhs=WT_sb[b][:, l * H:(l + 1) * H],
                                         start=True, stop=True)
                    stg = xpool.tile([128, 4, 128], bf, tag="stg", name="stg")
                    nc.vector.tensor_copy(out=stg[:, :nt, :],
                                          in_=ps2[:, :nt, :])
                    nc.sync.dma_start(
                        out=xw_bounce[pp][c0 * 128:c0 * 128 + cw, :].rearrange(
                            "(t p) h -> p t h", p=128),
                        in_=stg[:, :nt, :])
                nc.gpsimd.collective_compute(
                    "AllGather", AOT.bypass,
                    replica_groups=[list(range(N_CORES))],
                    ins=[xw_bounce[pp][:, :].opt()],
                    outs=[xw_full[pp][:, :].opt()],
                )

            def phase_b(k):
                b, l = steps[k]
                pp = k % 2
                gidx_t = mpool.tile([128, TTmax * 8], i16, tag="gidx",
                                    name="gidx")
                nc.sync.dma_start(out=gidx_t[:, :TT[b] * 8],
                                  in_=inp[f"{b}_gidx"][:, :])
                nrm_t = mpool.tile([128, TTmax], f32, tag="nrm", name="nrm")
                nc.sync.dma_start(out=nrm_t[:, :TT[b]],
                                  in_=inp[f"{b}_nrm"][:, :])
                dwin_t = mpool.tile([128, TTmax], f32, tag="dwin", name="dwin")
                nc.sync.dma_start(out=dwin_t[:, :TT[b]],
                                  in_=inp[f"{b}_dwin"][:, :])
                calls, sbbi_total = sched[b]
                calls_by_sb = {}
                for (sb, q, tl) in calls:
                    calls_by_sb.setdefault(sb, []).append((q, tl))
                qcnt = 0
                for sb in range(NSB):
                    nb = min(4, NBLK - sb * 4)
                    ps_bi = [pspool.tile([128, 512], f32, tag=f"psb{i}",
                                         name=f"psb{i}", bufs=1)
                             for i in range(nb)]
                    seen = [0, 0, 0, 0]
                    for (q, tl) in calls_by_sb[sb]:
                        if not tl:
                            continue
                        ntile = len(tl)
                        gt0 = tl[0][1]
                        C = cpool.tile([128, 32, 128], bf, tag="C", name="C")
                        nc.gpsimd.dma_gather(
                            C[:, :ntile, :],
                            xw_full[pp][q * NQ:(q + 1) * NQ, :],
                            gidx_t[:, gt0 * 8:(gt0 + ntile) * 8],
                            ntile * 128, ntile * 128, H,
                            single_packet=False, queue_num=qcnt % 4)
                        qcnt += 1
                        for i, (blk, gt) in enumerate(tl):
                            bi = blk - sb * 4
                            S = spool.tile([128, 128], bf, tag="S", name="S")
                            nc.vector.tensor_scalar(
                                out=S[:], in0=iota_b[:],
                                scalar1=dwin_t[:, gt:gt + 1],
                                scalar2=nrm_t[:, gt:gt + 1],
                                op0=AOT.is_equal, op1=AOT.mult)
                            first = seen[bi] == 0
                            last = seen[bi] == sbbi_total[sb][bi] - 1
                            nc.tensor.matmul(out=ps_bi[bi][:, 0:128],
                                             lhsT=S[:], rhs=C[:, i, :],
                                             start=bool(first),
                                             stop=bool(last))
                            seen[bi] += 1
                    _update(b, l, sb, ps_bi, nb)

            def _update(b, l, sb, ps_bi, nb):
                for bi in range(nb):
                    t1 = upool.tile([128, 128], f32, tag="t1", name="t1")
                    nc.vector.tensor_tensor(
                        out=t1[:], in0=ps_bi[bi][:, 0:128],
                        in1=bias_rep[(b, l)][:], op=AOT.add)
                    t2 = upool.tile([128, 128], f32, tag="t2", name="t2")
                    nc.scalar.activation(out=t2[:], in_=t1[:],
                                         func=ACT.Relu, bias=0.0, scale=1.0)
                    nc.vector.tensor_tensor(out=t2[:], in0=t2[:],
                                            in1=A_rep[b][:], op=AOT.mult)
                    nc.vector.tensor_tensor(out=t2[:], in0=t2[:],
                                            in1=B_rep[b][:], op=AOT.add)
                    nc.vector.tensor_tensor(
                        out=h_sb[b][:, sb * 4 + bi, :],
                        in0=h_sb[b][:, sb * 4 + bi, :],
                        in1=t2[:], op=AOT.add)

            for k in range(min(2, len(steps))):
                phase_a(k)
            for k in range(2, len(steps)):
                phase_b(k - 2)
                phase_a(k)
            for k in range(max(0, len(steps) - 2), len(steps)):
                phase_b(k)

            # ---- back: fuse + BN + head (node-major) ----
            lin_b = meta["lin_b"]
            for c0 in range(0, NBLK, 4):
                nt = min(4, NBLK - c0)
                hTs = {}
                for bb in _BRANCHES:
                    pst = psa.tile([128, 4, 128], f32, tag="pst", name="pst")
                    for t in range(nt):
                        nc.tensor.matmul(out=pst[:, t, :],
                                         lhsT=h_sb[bb][:, c0 + t, :],
                                         rhs=ident[:], is_transpose=True,
                                         start=True, stop=True)
                    hT = xpool.tile([128, 4, 128], f32, tag=f"hT_{bb}",
                                    name=f"hT_{bb}")
                    nc.scalar.activation(out=hT[:, :nt, :], in_=pst[:, :nt, :],
                                         func=ACT.Copy, bias=0.0, scale=1.0)
                    hTs[bb] = hT
                ps = psa.tile([128, 4, 128], f32, tag="ps2", name="ps2")
                for t in range(nt):
                    nc.tensor.matmul(out=ps[:, t, :], lhsT=hTs["call"][:, t, :],
                                     rhs=Wf1T_sb[:], start=True, stop=False)
                    nc.tensor.matmul(out=ps[:, t, :], lhsT=hTs["loc"][:, t, :],
                                     rhs=Wf2T_sb[:], start=False, stop=True)
                hf = upool.tile([128, 4, 128], f32, tag="ft", name="ft")
                nc.vector.tensor_tensor(
                    out=hf[:, :nt, :], in0=ps[:, :nt, :],
                    in1=fus_b_rep[:, None, :].to_broadcast([128, nt, 128]),
                    op=AOT.add)
                nc.vector.tensor_scalar(out=hf[:, :nt, :], in0=hf[:, :nt, :],
                                        scalar1=0.0, scalar2=None, op0=AOT.max)
                nc.vector.tensor_tensor(
                    out=hf[:, :nt, :], in0=hf[:, :nt, :],
                    in1=Af_rep[:, None, :].to_broadcast([128, nt, 128]),
                    op=AOT.mult)
                nc.vector.tensor_tensor(
                    out=hf[:, :nt, :], in0=hf[:, :nt, :],
                    in1=Bf_rep[:, None, :].to_broadcast([128, nt, 128]),
                    op=AOT.add)
                nc.vector.tensor_tensor(
                    out=hf[:, :nt, :], in0=hf[:, :nt, :],
                    in1=linW_rep[:, None, :].to_broadcast([128, nt, 128]),
                    op=AOT.mult)
                yred = upool.tile([128, 4], f32, tag="yred", name="yred")
                import concourse.mybir as _mb
                nc.vector.tensor_reduce(
                    out=yred[:, :nt], in_=hf[:, :nt, :],
                    axis=_mb.AxisListType.X, op=AOT.add)
                ycl = upool.tile([128, 4], f32, tag="ycl", name="ycl")
                nc.vector.tensor_scalar(out=ycl[:, :nt], in0=yred[:, :nt],
                                        scalar1=lin_b, scalar2=-10.0,
                                        op0=AOT.add, op1=AOT.max)
                nc.vector.tensor_scalar(out=ycl[:, :nt], in0=ycl[:, :nt],
                                        scalar1=10.0, scalar2=None,
                                        op0=AOT.min)
                nc.sync.dma_start(out=y_out[:, c0:c0 + nt], in_=ycl[:, :nt])

            if os.environ.get("GCN_DEBUG_H") == "1":
                for b in _BRANCHES:
                    dbg = nc.dram_tensor(f"dbg_h_{b}", [128, NBLK * 128], f32,
                                         kind="ExternalOutput")
                    nc.sync.dma_start(
                        out=dbg[:, :],
                        in_=h_sb[b][:].rearrange("p t h -> p (t h)"))

    nc.compile()
    return nc


# ----------------------------------------------------------------------------
# Entry point
# ----------------------------------------------------------------------------

def kernel(**inputs) -> np.ndarray:
    from concourse.bass_utils import run_bass_kernel_spmd

    meta, per_core_maps, perm = _host_prep(inputs)
    nc = _build_program(meta)
    trace = os.environ.get("GCN_TRACE", "") == "1"
    kw = {}
    if trace:
        kw = dict(trace=True)
    res = run_bass_kernel_spmd(nc, per_core_maps,
                               core_ids=list(range(N_CORES)), **kw)
    if trace:
        kernel.last_exec_time_ns = res.exec_time_ns
        kernel.last_trace = (res.instructions_and_trace[1]
                             if res.instructions_and_trace else None)
    kernel.last_results = res.results
    n_nodes = meta["n_nodes"]
    NBLK = meta["NBLK"]
    # y[p, t] = node t*128 + p (per core)
    y_pad = np.concatenate(
        [np.asarray(res.results[c]["y"]).T.reshape(-1) for c in range(N_CORES)])
    out = y_pad[perm[:n_nodes]].astype(np.float32).reshape(n_nodes, 1)
    return out


# revision 13
# speedup vs baseline: 2.3319x; 1.1847x over previous
"""Trainium2 Bass kernel for nn_EnhancedDualGCN (dual 3-layer GCN, N=100k, E=1.6M).

Node-sharded across 8 NeuronCores, v2 (scatter-free):
  - Host renumbers nodes (balanced blocks), pads to N_PAD=784*128; core c owns
    NL=12544 consecutive new-ids. h kept NODE-major in SBUF ([128, 98, 128]).
  - Per GCN layer: xw = h @ W^T per shard (PE transpose of h blocks + matmul),
    AllGather the full [N_PAD, H] bf16 table.
  - Edges sorted by (dst-superblock, src-quadrant, dst-block); 128-edge tiles.
    Per tile: C = dma_gather(xw_full_quadrant, src) [128e, H] (4 SWDGE queues);
    S[e, d] = nrm[e] * (dwin[e] == d) built by one DVE tensor_scalar;
    PE matmul lhsT=S rhs=C accumulates node-major [dst, H] into the
    superblock's PSUM tile. No dma_scatter: PSUM accumulation replaces it.
  - Update: h += BN(relu(psum + bias)) with feature-wise BN applied through
    host-replicated [128,128] constant tiles (node-major friendly).
"""

import os
import sys

sys.path.insert(0, "/opt/trn_rl_repo")

import numpy as np

H = 128
L = 3
N_CORES = 8
BN_EPS = 1e-5
F32 = np.float32
BF16 = None  # set lazily (ml_dtypes)

_BRANCHES = ("call", "loc")


# ----------------------------------------------------------------------------
# Host-side preprocessing
# ----------------------------------------------------------------------------

def _branch_edges(ei, ew, n_nodes):
    src = np.concatenate([ei[0], np.arange(n_nodes, dtype=np.int64)])
    dst = np.concatenate([ei[1], np.arange(n_nodes, dtype=np.int64)])
    w = np.concatenate([ew, np.ones(n_nodes, ew.dtype)]).astype(F32)
    deg = np.zeros(n_nodes, F32)
    np.add.at(deg, dst, w)
    dis = np.where(deg > 0, 1.0 / np.sqrt(deg), 0.0).astype(F32)
    norm = (dis[src] * w * dis[dst]).astype(F32)
    return src, dst, norm


def _build_perm(deg_sum, n_pad):
    order = np.argsort(-deg_sum, kind="stable")
    n_blocks = n_pad // 128
    perm = np.empty(n_pad, dtype=np.int64)
    perm[order] = (np.arange(n_pad) % n_blocks) * 128 + np.arange(n_pad) // n_blocks
    return perm


def _wrap16(vals):
    """[n] int -> [128, n//16] int16: 16-partition wrap, replicated across the
    8 GPSIMD core stripes (HW requirement)."""
    pat = vals.reshape(-1, 16).T.astype(np.int16)
    return np.tile(pat, (8, 1))


def _prep_branch(ei, ew, perm, n_nodes, n_pad):
    """Edge metadata for one branch.

    Returns (T, per_core) where T[q][blk] = uniform (max-over-core) tile count
    and per_core[c] = dict(gidx [128, TT*8] i16, nrm [128, TT] bf16,
    dwin [128, TT] bf16) laid out in (sb, q, blk, tile) order.
    """
    NL = n_pad // N_CORES
    NQ = n_pad // 4
    NBLK = NL // 128
    src, dst, norm = _branch_edges(ei, ew, n_nodes)
    nsrc = perm[src]
    ndst = perm[dst]
    core = ndst // NL
    dstl = ndst % NL
    q = nsrc // NQ
    srcq = (nsrc % NQ).astype(np.int32)
    blk = dstl // 128

    # counts per (core, q, blk)
    counts = np.zeros((N_CORES, 4, NBLK), np.int64)
    np.add.at(counts, (core, q, blk), 1)
    tiles = -(-counts // 128)  # ceil
    T = tiles.max(axis=0)  # [4, NBLK] uniform tile counts

    n_sb = (NBLK + 3) // 4
    # processing order: sb, q, blk
    order_blocks = []
    for sb in range(n_sb):
        for qq in range(4):
            for b in range(sb * 4, min(sb * 4 + 4, NBLK)):
                order_blocks.append((qq, b))
    TT = int(sum(T[qq][b] for qq, b in order_blocks))

    per_core = []
    for c in range(N_CORES):
        m = core == c
        # sort this core's edges by (q, blk, dstl) -> group per (q, blk)
        o = np.lexsort((dstl[m], blk[m], q[m]))
        cs = srcq[m][o]
        cn = norm[m][o]
        cd = (dstl[m][o] % 128).astype(F32)
        cq = q[m][o]
        cb = blk[m][o]
        # start offset of each (q, blk) group
        gidx = np.zeros(TT * 128, np.int32)
        nrm_a = np.zeros(TT * 128, F32)
        dwin_a = np.zeros(TT * 128, F32)
        bounds = np.searchsorted(cq * NBLK + cb, np.arange(4 * NBLK + 1),
                                 side="left",
                                 sorter=None)
        pos = 0
        for qq, b in order_blocks:
            lo, hi = bounds[qq * NBLK + b], bounds[qq * NBLK + b + 1]
            cnt = hi - lo
            t_need = T[qq][b]
            gidx[pos:pos + cnt] = cs[lo:hi]
            nrm_a[pos:pos + cnt] = cn[lo:hi]
            dwin_a[pos:pos + cnt] = cd[lo:hi]
            pos += t_need * 128
        assert pos == TT * 128
        import ml_dtypes
        per_core.append({
            "gidx": _wrap16(gidx),
            "nrm": np.ascontiguousarray(
                nrm_a.reshape(TT, 128).T).astype(ml_dtypes.bfloat16),
            "dwin": np.ascontiguousarray(
                dwin_a.reshape(TT, 128).T).astype(ml_dtypes.bfloat16),
        })
    return T, TT, per_core


def _affine_bn(p):
    g, b, m, v = [np.asarray(x, F32) for x in (p[0], p[1], p[2], p[3])]
    A = (g / np.sqrt(v + BN_EPS)).astype(F32)
    B = (b - m * A).astype(F32)
    return A, B


def _rep(v):
    """[H] -> [128, H] replicated f32 const tile."""
    return np.ascontiguousarray(np.tile(np.asarray(v, F32).reshape(1, H),
                                        (128, 1)))


def _host_prep(inputs):
    n_nodes = inputs["x"].shape[0]
    n_pad = ((n_nodes + N_CORES * 128 - 1) // (N_CORES * 128)) * (N_CORES * 128)
    NL = n_pad // N_CORES

    ei_c = np.asarray(inputs["edge_index_call"], np.int64)
    ei_l = np.asarray(inputs["edge_index_loc"], np.int64)
    deg_sum = np.zeros(n_pad, np.int64)
    np.add.at(deg_sum[:n_nodes], ei_c[1], 1)
    np.add.at(deg_sum[:n_nodes], ei_l[1], 1)
    perm = _build_perm(deg_sum, n_pad)

    T = {}
    TT = {}
    per_core_maps = [dict() for _ in range(N_CORES)]
    for b, ei, ew in (("call", ei_c, inputs["edge_weight_call"]),
                      ("loc", ei_l, inputs["edge_weight_loc"])):
        Tb, TTb, pc = _prep_branch(ei, np.asarray(ew, F32), perm, n_nodes,
                                   n_pad)
        T[b] = Tb
        TT[b] = TTb
        for c in range(N_CORES):
            per_core_maps[c][f"{b}_gidx"] = pc[c]["gidx"]
            per_core_maps[c][f"{b}_nrm"] = pc[c]["nrm"]
            per_core_maps[c][f"{b}_dwin"] = pc[c]["dwin"]

    # features (permuted, padded, feature-major)
    x = np.nan_to_num(np.asarray(inputs["x"], F32))
    emb = np.asarray(inputs["emb"], F32)
    x_pad = np.zeros((n_pad, x.shape[1]), F32)
    emb_pad = np.zeros((n_pad, emb.shape[1]), F32)
    x_pad[perm[:n_nodes]] = x
    emb_pad[perm[:n_nodes]] = emb
    for c in range(N_CORES):
        sl = slice(c * NL, (c + 1) * NL)
        per_core_maps[c]["xT"] = np.ascontiguousarray(x_pad[sl].T)
        per_core_maps[c]["embT"] = np.ascontiguousarray(emb_pad[sl].T)

    # weights (shared)
    comb_W = np.asarray(inputs["comb_W"], F32)
    Wc1 = comb_W[:, :emb.shape[1]]
    Wc2 = comb_W[:, emb.shape[1]:]
    Wx = Wc2 @ np.asarray(inputs["ft_W"], F32)  # [H, IN] (ft_b == 0)
    common = {
        "WxT": np.ascontiguousarray(Wx.T),
        "Wc1T": np.ascontiguousarray(Wc1.T),
        "comb_b_rep": _rep(inputs["comb_b"]),
    }
    for b in _BRANCHES:
        Ws = np.asarray(inputs["call_W" if b == "call" else "loc_W"], F32)
        bs = np.asarray(inputs["call_b" if b == "call" else "loc_b"], F32)
        A, B = _affine_bn(np.asarray(inputs[f"bn_{b}"], F32))
        WT = np.concatenate([Ws[l].T for l in range(L)], axis=1)  # [H, L*H]
        common[f"{b}_WT"] = np.ascontiguousarray(WT)
        for l in range(L):
            common[f"{b}{l}_bias_rep"] = _rep(bs[l])
        common[f"{b}_A_rep"] = _rep(A)
        common[f"{b}_B_rep"] = _rep(B)
    fus_W = np.asarray(inputs["fus_W"], F32)
    Af, Bf = _affine_bn(np.asarray(inputs["bn_fus"], F32))
    common.update({
        "Wf1T": np.ascontiguousarray(fus_W[:, :H].T),
        "Wf2T": np.ascontiguousarray(fus_W[:, H:].T),
        "fus_b_rep": _rep(inputs["fus_b"]),
        "Af_rep": _rep(Af),
        "Bf_rep": _rep(Bf),
        "linW_rep": _rep(np.asarray(inputs["lin_W"], F32).reshape(H)),
    })
    lin_b = float(np.asarray(inputs["lin_b"], F32).reshape(-1)[0])

    meta = {
        "n_nodes": n_nodes,
        "n_pad": n_pad,
        "NL": NL,
        "NQ": n_pad // 4,
        "NBLK": NL // 128,
        "T": T,
        "TT": TT,
        "in_dim": x.shape[1],
        "emb_dim": emb.shape[1],
        "lin_b": lin_b,
    }
    for c in range(N_CORES):
        per_core_maps[c].update(common)
    return meta, per_core_maps, perm


# ----------------------------------------------------------------------------
# Device program
# ----------------------------------------------------------------------------

def _build_program(meta):
    import contextlib

    import concourse.bass as bass
    import concourse.bacc as bacc
    import concourse.mybir as mybir
    import concourse.tile as tile
    from concourse.masks import make_identity

    f32 = mybir.dt.float32
    bf = mybir.dt.bfloat16
    i16 = mybir.dt.int16
    AOT = mybir.AluOpType
    ACT = mybir.ActivationFunctionType

    NL = meta["NL"]
    NQ = meta["NQ"]
    NP = meta["n_pad"]
    NBLK = meta["NBLK"]
    NSB = (NBLK + 3) // 4
    IN_DIM = meta["in_dim"]
    EMB_DIM = meta["emb_dim"]
    T = meta["T"]
    TT = meta["TT"]

    nc = bacc.Bacc(None, num_devices=N_CORES, num_swdge_queues=4)

    inp = {}

    def ext(name, shape, dt=f32):
        inp[name] = nc.dram_tensor(name, list(shape), dt, kind="ExternalInput")
        return inp[name]

    ext("xT", [IN_DIM, NL])
    ext("embT", [EMB_DIM, NL])
    ext("WxT", [IN_DIM, H])
    ext("Wc1T", [EMB_DIM, H])
    ext("comb_b_rep", [128, H])
    for b in _BRANCHES:
        ext(f"{b}_WT", [H, L * H])
        for l in range(L):
            ext(f"{b}{l}_bias_rep", [128, H])
        ext(f"{b}_A_rep", [128, H])
        ext(f"{b}_B_rep", [128, H])
        ext(f"{b}_gidx", [128, TT[b] * 8], i16)
        ext(f"{b}_nrm", [128, TT[b]], bf)
        ext(f"{b}_dwin", [128, TT[b]], bf)
    ext("Wf1T", [H, H])
    ext("Wf2T", [H, H])
    ext("fus_b_rep", [128, H])
    ext("Af_rep", [128, H])
    ext("Bf_rep", [128, H])
    ext("linW_rep", [128, H])
    y_out = nc.dram_tensor("y", [128, NBLK], f32, kind="ExternalOutput")

    xw_bounce = [nc.dram_tensor(f"xw_bounce{i}", [NL, H], bf)
                 for i in range(2)]
    xw_full = [nc.dram_tensor(f"xw_full{i}", [NP, H], bf, addr_space="Shared")
               for i in range(2)]

    steps = [(b, l) for l in range(L) for b in _BRANCHES]
    steps = steps[:int(os.environ.get("GCN_STEPS", str(len(steps))))]

    # per-branch codegen schedule: list of (sb, q, blk_list, tile_gt_list)
    # and per (sb, bi): total tiles (for start/stop flags)
    sched = {}
    for b in _BRANCHES:
        Tb = T[b]
        calls = []
        sbbi_total = np.zeros((NSB, 4), np.int64)
        gt = 0
        for sb in range(NSB):
            blks = list(range(sb * 4, min(sb * 4 + 4, NBLK)))
            for q in range(4):
                tl = []
                for blk in blks:
                    for _ in range(int(Tb[q][blk])):
                        tl.append((blk, gt))
                        gt += 1
                        sbbi_total[sb][blk - sb * 4] += 1
                calls.append((sb, q, tl))
        assert gt == TT[b]
        sched[b] = (calls, sbbi_total)
    max_call = max(len(tl) for b in _BRANCHES
                   for (sb, q, tl) in sched[b][0])

    with tile.TileContext(nc) as tc:
        with contextlib.ExitStack() as ctx:
            konst = ctx.enter_context(tc.tile_pool(name="konst", bufs=1))
            hpool = ctx.enter_context(tc.tile_pool(name="hpool", bufs=1))
            mpool = ctx.enter_context(tc.tile_pool(name="mpool", bufs=1))
            cpool = ctx.enter_context(tc.tile_pool(name="cpool", bufs=2))
            spool = ctx.enter_context(tc.tile_pool(name="spool", bufs=2))
            upool = ctx.enter_context(tc.tile_pool(name="upool", bufs=2))
            xpool = ctx.enter_context(tc.tile_pool(name="xpool", bufs=2))
            pspool = ctx.enter_context(
                tc.tile_pool(name="ps", bufs=3, space="PSUM"))
            psa = ctx.enter_context(
                tc.tile_pool(name="psa", bufs=2, space="PSUM"))

            # ---- constants ----
            ident = konst.tile([128, 128], f32, tag="ident", name="ident")
            make_identity(nc, ident[:])
            iota_i = konst.tile([128, 128], mybir.dt.int32, tag="iota_i",
                                name="iota_i")
            nc.gpsimd.iota(iota_i[:], pattern=[[1, 128]], base=0,
                           channel_multiplier=0)
            iota_b = konst.tile([128, 128], bf, tag="iota_b", name="iota_b")
            nc.vector.tensor_copy(out=iota_b[:], in_=iota_i[:])

            def kload(name, shape, dt=f32):
                t = konst.tile(shape, dt, tag=name, name=name)
                nc.sync.dma_start(out=t[:], in_=inp[name][:, :])
                return t

            WT_sb = {b: kload(f"{b}_WT", [H, L * H]) for b in _BRANCHES}
            bias_rep = {(b, l): kload(f"{b}{l}_bias_rep", [128, H])
                        for b in _BRANCHES for l in range(L)}
            A_rep = {b: kload(f"{b}_A_rep", [128, H]) for b in _BRANCHES}
            B_rep = {b: kload(f"{b}_B_rep", [128, H]) for b in _BRANCHES}
            WxT_sb = kload("WxT", [IN_DIM, H])
            Wc1T_sb = kload("Wc1T", [EMB_DIM, H])
            comb_b_rep = kload("comb_b_rep", [128, H])
            Wf1T_sb = kload("Wf1T", [H, H])
            Wf2T_sb = kload("Wf2T", [H, H])
            fus_b_rep = kload("fus_b_rep", [128, H])
            Af_rep = kload("Af_rep", [128, H])
            Bf_rep = kload("Bf_rep", [128, H])
            linW_rep = kload("linW_rep", [128, H])

            TTmax = max(TT[b] for b in _BRANCHES)

            h_sb = {b: hpool.tile([128, NBLK, 128], f32, tag=f"h_{b}",
                                  name=f"h_{b}") for b in _BRANCHES}

            # ---- front: h0 = relu(emb@Wc1^T + x@Wx^T + comb_b), node-major --
            with tc.tile_pool(name="front", bufs=1) as fpool:
                for c0 in range(0, NBLK, 4):
                    nt = min(4, NBLK - c0)
                    n0 = c0 * 128
                    cw = nt * 128
                    xT_sb = fpool.tile([IN_DIM, 4 * 128], f32, tag="xT",
                                       name="xT")
                    embT_sb = fpool.tile([EMB_DIM, 4 * 128], f32, tag="embT",
                                         name="embT")
                    nc.sync.dma_start(out=xT_sb[:, :cw],
                                      in_=inp["xT"][:, n0:n0 + cw])
                    nc.sync.dma_start(out=embT_sb[:, :cw],
                                      in_=inp["embT"][:, n0:n0 + cw])
                    ps = psa.tile([128, 4, 128], f32, tag="pst", name="pst")
                    for t in range(nt):
                        nc.tensor.matmul(out=ps[:, t, :],
                                         lhsT=xT_sb[:, t * 128:(t + 1) * 128],
                                         rhs=WxT_sb[:],
                                         start=True, stop=False)
                        nc.tensor.matmul(out=ps[:, t, :],
                                         lhsT=embT_sb[:, t * 128:(t + 1) * 128],
                                         rhs=Wc1T_sb[:],
                                         start=False, stop=True)
                    tmp = upool.tile([128, 4, 128], f32, tag="ft",
                                     name="ft")
                    nc.vector.tensor_tensor(
                        out=tmp[:, :nt, :], in0=ps[:, :nt, :],
                        in1=comb_b_rep[:, None, :].to_broadcast(
                            [128, nt, 128]),
                        op=AOT.add)
                    nc.vector.tensor_scalar(
                        out=h_sb["call"][:, c0:c0 + nt, :], in0=tmp[:, :nt, :],
                        scalar1=0.0, scalar2=None, op0=AOT.max)
                    nc.vector.tensor_copy(out=h_sb["loc"][:, c0:c0 + nt, :],
                                          in_=h_sb["call"][:, c0:c0 + nt, :])

            # ---- phases ----
            def phase_a(k):
                b, l = steps[k]
                pp = k % 2
                for c0 in range(0, NBLK, 4):
                    nt = min(4, NBLK - c0)
                    cw = nt * 128
                    pst = psa.tile([128, 4, 128], f32, tag="pst", name="pst")
                    for t in range(nt):
                        nc.tensor.matmul(out=pst[:, t, :],
                                         lhsT=h_sb[b][:, c0 + t, :],
                                         rhs=ident[:], is_transpose=True,
                                         start=True, stop=True)
                    hT = xpool.tile([128, 4, 128], f32, tag="hT", name="hT")
                    nc.scalar.activation(out=hT[:, :nt, :], in_=pst[:, :nt, :],
                                         func=ACT.Copy, bias=0.0, scale=1.0)
                    ps2 = psa.tile([128, 4, 128], f32, tag="ps2", name="ps2")
                    for t in range(nt):
                        nc.tensor.matmul(out=ps2[:, t, :],
                                         lhsT=hT[:, t, :],
                                         rhs=WT_sb[b][:, l * H:(l + 1) * H],
                                         start=True, stop=True)
                    stg = xpool.tile([128, 4, 128], bf, tag="stg", name="stg")
                    nc.vector.tensor_copy(out=stg[:, :nt, :],
                                          in_=ps2[:, :nt, :])
                    nc.sync.dma_start(
                        out=xw_bounce[pp][c0 * 128:c0 * 128 + cw, :].rearrange(
                            "(t p) h -> p t h", p=128),
                        in_=stg[:, :nt, :])
                nc.gpsimd.collective_compute(
                    "AllGather", AOT.bypass,
                    replica_groups=[list(range(N_CORES))],
                    ins=[xw_bounce[pp][:, :].opt()],
                    outs=[xw_full[pp][:, :].opt()],
                )

            def phase_b(k):
                b, l = steps[k]
                pp = k % 2
                gidx_t = mpool.tile([128, TTmax * 8], i16, tag="gidx",
                                    name="gidx")
                nc.sync.dma_start(out=gidx_t[:, :TT[b] * 8],
                                  in_=inp[f"{b}_gidx"][:, :])
                nrm_t = mpool.tile([128, TTmax], bf, tag="nrm", name="nrm")
                nc.sync.dma_start(out=nrm_t[:, :TT[b]],
                                  in_=inp[f"{b}_nrm"][:, :])
                dwin_t = mpool.tile([128, TTmax], bf, tag="dwin", name="dwin")
                nc.sync.dma_start(out=dwin_t[:, :TT[b]],
                                  in_=inp[f"{b}_dwin"][:, :])
                calls, sbbi_total = sched[b]
                calls_by_sb = {}
                for (sb, q, tl) in calls:
                    calls_by_sb.setdefault(sb, []).append((q, tl))
                qcnt = 0
                for sb in range(NSB):
                    nb = min(4, NBLK - sb * 4)
                    ps_bi = [pspool.tile([128, 512], f32, tag=f"psb{i}",
                                         name=f"psb{i}", bufs=1)
                             for i in range(nb)]
                    seen = [0, 0, 0, 0]
                    for (q, tl) in calls_by_sb[sb]:
                        if not tl:
                            continue
                        ntile = len(tl)
                        gt0 = tl[0][1]
                        C = cpool.tile([128, max_call, 128], bf, tag="C",
                                       name="C")
                        nc.gpsimd.dma_gather(
                            C[:, :ntile, :],
                            xw_full[pp][q * NQ:(q + 1) * NQ, :],
                            gidx_t[:, gt0 * 8:(gt0 + ntile) * 8],
                            ntile * 128, ntile * 128, H,
                            single_packet=False, queue_num=qcnt % 4)
                        qcnt += 1
                        Sg = spool.tile([128, max_call, 128], bf, tag="Sg",
                                        name="Sg")
                        nc.vector.tensor_tensor(
                            out=Sg[:, :ntile, :],
                            in0=iota_b[:, None, :].to_broadcast(
                                [128, ntile, 128]),
                            in1=dwin_t[:, gt0:gt0 + ntile, None].to_broadcast(
                                [128, ntile, 128]),
                            op=AOT.is_equal)
                        nc.vector.tensor_tensor(
                            out=Sg[:, :ntile, :], in0=Sg[:, :ntile, :],
                            in1=nrm_t[:, gt0:gt0 + ntile, None].to_broadcast(
                                [128, ntile, 128]),
                            op=AOT.mult)
                        for i, (blk, gt) in enumerate(tl):
                            bi = blk - sb * 4
                            first = seen[bi] == 0
                            last = seen[bi] == sbbi_total[sb][bi] - 1
                            nc.tensor.matmul(out=ps_bi[bi][:, 0:128],
                                             lhsT=Sg[:, i, :], rhs=C[:, i, :],
                                             start=bool(first),
                                             stop=bool(last))
                            seen[bi] += 1
                    _update(b, l, sb, ps_bi, nb)

            def _update(b, l, sb, ps_bi, nb):
                for bi in range(nb):
                    t1 = upool.tile([128, 128], f32, tag="t1", name="t1")
                    nc.vector.tensor_tensor(
                        out=t1[:], in0=ps_bi[bi][:, 0:128],
                        in1=bias_rep[(b, l)][:], op=AOT.add)
                    t2 = upool.tile([128, 128], f32, tag="t2", name="t2")
                    nc.scalar.activation(out=t2[:], in_=t1[:],
                                         func=ACT.Relu, bias=0.0, scale=1.0)
                    nc.vector.tensor_tensor(out=t2[:], in0=t2[:],
                                            in1=A_rep[b][:], op=AOT.mult)
                    nc.vector.tensor_tensor(out=t2[:], in0=t2[:],
                                            in1=B_rep[b][:], op=AOT.add)
                    nc.vector.tensor_tensor(
                        out=h_sb[b][:, sb * 4 + bi, :],
                        in0=h_sb[b][:, sb * 4 + bi, :],
                        in1=t2[:], op=AOT.add)

            for k in range(min(2, len(steps))):
                phase_a(k)
            for k in range(2, len(steps)):
                phase_b(k - 2)
                phase_a(k)
            for k in range(max(0, len(steps) - 2), len(steps)):
                phase_b(k)

            # ---- back: fuse + BN + head (node-major) ----
            lin_b = meta["lin_b"]
            for c0 in range(0, NBLK, 4):
                nt = min(4, NBLK - c0)
                hTs = {}
                for bb in _BRANCHES:
                    pst = psa.tile([128, 4, 128], f32, tag="pst", name="pst")
                    for t in range(nt):
                        nc.tensor.matmul(out=pst[:, t, :],
                                         lhsT=h_sb[bb][:, c0 + t, :],
                                         rhs=ident[:], is_transpose=True,
                                         start=True, stop=True)
                    hT = xpool.tile([128, 4, 128], f32,
                                    tag=("hT" if bb == "call" else "hT_loc"),
                                    name="hTb")
                    nc.scalar.activation(out=hT[:, :nt, :], in_=pst[:, :nt, :],
                                         func=ACT.Copy, bias=0.0, scale=1.0)
                    hTs[bb] = hT
                ps = psa.tile([128, 4, 128], f32, tag="ps2", name="ps2")
                for t in range(nt):
                    nc.tensor.matmul(out=ps[:, t, :], lhsT=hTs["call"][:, t, :],
                                     rhs=Wf1T_sb[:], start=True, stop=False)
                    nc.tensor.matmul(out=ps[:, t, :], lhsT=hTs["loc"][:, t, :],
                                     rhs=Wf2T_sb[:], start=False, stop=True)
                hf = upool.tile([128, 4, 128], f32, tag="ft", name="ft")
                nc.vector.tensor_tensor(
                    out=hf[:, :nt, :], in0=ps[:, :nt, :],
                    in1=fus_b_rep[:, None, :].to_broadcast([128, nt, 128]),
                    op=AOT.add)
                nc.vector.tensor_scalar(out=hf[:, :nt, :], in0=hf[:, :nt, :],
                                        scalar1=0.0, scalar2=None, op0=AOT.max)
                nc.vector.tensor_tensor(
                    out=hf[:, :nt, :], in0=hf[:, :nt, :],
                    in1=Af_rep[:, None, :].to_broadcast([128, nt, 128]),
                    op=AOT.mult)
                nc.vector.tensor_tensor(
                    out=hf[:, :nt, :], in0=hf[:, :nt, :],
                    in1=Bf_rep[:, None, :].to_broadcast([128, nt, 128]),
                    op=AOT.add)
                nc.vector.tensor_tensor(
                    out=hf[:, :nt, :], in0=hf[:, :nt, :],
                    in1=linW_rep[:, None, :].to_broadcast([128, nt, 128]),
                    op=AOT.mult)
                yred = upool.tile([128, 4], f32, tag="yred", name="yred")
                import concourse.mybir as _mb
                nc.vector.tensor_reduce(
                    out=yred[:, :nt], in_=hf[:, :nt, :],
                    axis=_mb.AxisListType.X, op=AOT.add)
                ycl = upool.tile([128, 4], f32, tag="ycl", name="ycl")
                nc.vector.tensor_scalar(out=ycl[:, :nt], in0=yred[:, :nt],
                                        scalar1=lin_b, scalar2=-10.0,
                                        op0=AOT.add, op1=AOT.max)
                nc.vector.tensor_scalar(out=ycl[:, :nt], in0=ycl[:, :nt],
                                        scalar1=10.0, scalar2=None,
                                        op0=AOT.min)
                nc.sync.dma_start(out=y_out[:, c0:c0 + nt], in_=ycl[:, :nt])

            if os.environ.get("GCN_DEBUG_H") == "1":
                for b in _BRANCHES:
                    dbg = nc.dram_tensor(f"dbg_h_{b}", [128, NBLK * 128], f32,
                                         kind="ExternalOutput")
                    nc.sync.dma_start(
                        out=dbg[:, :],
                        in_=h_sb[b][:].rearrange("p t h -> p (t h)"))

    nc.compile()
    return nc


# ----------------------------------------------------------------------------
# Entry point
# ----------------------------------------------------------------------------

def kernel(**inputs) -> np.ndarray:
    from concourse.bass_utils import run_bass_kernel_spmd

    meta, per_core_maps, perm = _host_prep(inputs)
    nc = _build_program(meta)
    trace = os.environ.get("GCN_TRACE", "") == "1"
    kw = {}
    if trace:
        kw = dict(trace=True)
    res = run_bass_kernel_spmd(nc, per_core_maps,
                               core_ids=list(range(N_CORES)), **kw)
    if trace:
        kernel.last_exec_time_ns = res.exec_time_ns
        kernel.last_trace = (res.instructions_and_trace[1]
                             if res.instructions_and_trace else None)
    kernel.last_results = res.results
    n_nodes = meta["n_nodes"]
    NBLK = meta["NBLK"]
    # y[p, t] = node t*128 + p (per core)
    y_pad = np.concatenate(
        [np.asarray(res.results[c]["y"]).T.reshape(-1) for c in range(N_CORES)])
    out = y_pad[perm[:n_nodes]].astype(np.float32).reshape(n_nodes, 1)
    return out


# revision 14
# speedup vs baseline: 3.2335x; 1.3867x over previous
"""Trainium2 Bass kernel for nn_EnhancedDualGCN (dual 3-layer GCN, N=100k, E=1.6M).

Node-sharded across 8 NeuronCores, v2 (scatter-free):
  - Host renumbers nodes (balanced blocks), pads to N_PAD=784*128; core c owns
    NL=12544 consecutive new-ids. h kept NODE-major in SBUF ([128, 98, 128]).
  - Per GCN layer: xw = h @ W^T per shard (PE transpose of h blocks + matmul),
    AllGather the full [N_PAD, H] bf16 table.
  - Edges sorted by (dst-superblock, src-quadrant, dst-block); 128-edge tiles.
    Per tile: C = dma_gather(xw_full_quadrant, src) [128e, H] (4 SWDGE queues);
    S[e, d] = nrm[e] * (dwin[e] == d) built by one DVE tensor_scalar;
    PE matmul lhsT=S rhs=C accumulates node-major [dst, H] into the
    superblock's PSUM tile. No dma_scatter: PSUM accumulation replaces it.
  - Update: h += BN(relu(psum + bias)) with feature-wise BN applied through
    host-replicated [128,128] constant tiles (node-major friendly).
"""

import os
import sys

sys.path.insert(0, "/opt/trn_rl_repo")

import numpy as np

H = 128
L = 3
N_CORES = 8
BN_EPS = 1e-5
F32 = np.float32
BF16 = None  # set lazily (ml_dtypes)

_BRANCHES = ("call", "loc")


# ----------------------------------------------------------------------------
# Host-side preprocessing
# ----------------------------------------------------------------------------

def _branch_edges(ei, ew, n_nodes):
    src = np.concatenate([ei[0], np.arange(n_nodes, dtype=np.int64)])
    dst = np.concatenate([ei[1], np.arange(n_nodes, dtype=np.int64)])
    w = np.concatenate([ew, np.ones(n_nodes, ew.dtype)]).astype(F32)
    deg = np.zeros(n_nodes, F32)
    np.add.at(deg, dst, w)
    dis = np.where(deg > 0, 1.0 / np.sqrt(deg), 0.0).astype(F32)
    norm = (dis[src] * w * dis[dst]).astype(F32)
    return src, dst, norm


def _build_perm(deg_sum, n_pad):
    order = np.argsort(-deg_sum, kind="stable")
    n_blocks = n_pad // 128
    perm = np.empty(n_pad, dtype=np.int64)
    perm[order] = (np.arange(n_pad) % n_blocks) * 128 + np.arange(n_pad) // n_blocks
    return perm


def _wrap16(vals):
    """[n] int -> [128, n//16] int16: 16-partition wrap, replicated across the
    8 GPSIMD core stripes (HW requirement)."""
    pat = vals.reshape(-1, 16).T.astype(np.int16)
    return np.tile(pat, (8, 1))


def _prep_branch(ei, ew, perm, n_nodes, n_pad):
    """Edge metadata for one branch.

    Returns (T, per_core) where T[q][blk] = uniform (max-over-core) tile count
    and per_core[c] = dict(gidx [128, TT*8] i16, nrm [128, TT] bf16,
    dwin [128, TT] bf16) laid out in (sb, q, blk, tile) order.
    """
    NL = n_pad // N_CORES
    NQ = n_pad // 4
    NBLK = NL // 128
    src, dst, norm = _branch_edges(ei, ew, n_nodes)
    nsrc = perm[src]
    ndst = perm[dst]
    core = ndst // NL
    dstl = ndst % NL
    q = nsrc // NQ
    srcq = (nsrc % NQ).astype(np.int32)
    blk = dstl // 128

    # counts per (core, q, blk)
    counts = np.zeros((N_CORES, 4, NBLK), np.int64)
    np.add.at(counts, (core, q, blk), 1)
    tiles = -(-counts // 128)  # ceil
    T = tiles.max(axis=0)  # [4, NBLK] uniform tile counts

    n_sb = (NBLK + 3) // 4
    # processing order: sb, q, blk
    order_blocks = []
    for sb in range(n_sb):
        for qq in range(4):
            for b in range(sb * 4, min(sb * 4 + 4, NBLK)):
                order_blocks.append((qq, b))
    TT = int(sum(T[qq][b] for qq, b in order_blocks))

    per_core = []
    for c in range(N_CORES):
        m = core == c
        # sort this core's edges by (q, blk, dstl) -> group per (q, blk)
        o = np.lexsort((dstl[m], blk[m], q[m]))
        cs = srcq[m][o]
        cn = norm[m][o]
        cd = (dstl[m][o] % 128).astype(F32)
        cq = q[m][o]
        cb = blk[m][o]
        # start offset of each (q, blk) group
        gidx = np.zeros(TT * 128, np.int32)
        nrm_a = np.zeros(TT * 128, F32)
        dwin_a = np.zeros(TT * 128, F32)
        bounds = np.searchsorted(cq * NBLK + cb, np.arange(4 * NBLK + 1),
                                 side="left",
                                 sorter=None)
        pos = 0
        for qq, b in order_blocks:
            lo, hi = bounds[qq * NBLK + b], bounds[qq * NBLK + b + 1]
            cnt = hi - lo
            t_need = T[qq][b]
            gidx[pos:pos + cnt] = cs[lo:hi]
            nrm_a[pos:pos + cnt] = cn[lo:hi]
            dwin_a[pos:pos + cnt] = cd[lo:hi]
            pos += t_need * 128
        assert pos == TT * 128
        import ml_dtypes
        nrm_t = np.ascontiguousarray(nrm_a.reshape(TT, 128).T)  # [128, TT]
        dwin_t = np.ascontiguousarray(
            dwin_a.reshape(TT, 128).T).astype(np.int64)
        # S tiles: S_dev[p, gt*128 + d] = nrm[p, gt] iff dwin[p, gt] == d
        S_dev = np.zeros((128, TT * 128), np.float32)
        base = np.arange(TT, dtype=np.int64) * 128
        for p in range(128):
            S_dev[p, base + dwin_t[p]] = nrm_t[p]
        per_core.append({
            "gidx": _wrap16(gidx),
            "S": S_dev.astype(ml_dtypes.bfloat16),
        })
    return T, TT, per_core


def _affine_bn(p):
    g, b, m, v = [np.asarray(x, F32) for x in (p[0], p[1], p[2], p[3])]
    A = (g / np.sqrt(v + BN_EPS)).astype(F32)
    B = (b - m * A).astype(F32)
    return A, B


def _rep(v):
    """[H] -> [128, H] replicated f32 const tile."""
    return np.ascontiguousarray(np.tile(np.asarray(v, F32).reshape(1, H),
                                        (128, 1)))


def _host_prep(inputs):
    n_nodes = inputs["x"].shape[0]
    n_pad = ((n_nodes + N_CORES * 128 - 1) // (N_CORES * 128)) * (N_CORES * 128)
    NL = n_pad // N_CORES

    ei_c = np.asarray(inputs["edge_index_call"], np.int64)
    ei_l = np.asarray(inputs["edge_index_loc"], np.int64)
    deg_sum = np.zeros(n_pad, np.int64)
    np.add.at(deg_sum[:n_nodes], ei_c[1], 1)
    np.add.at(deg_sum[:n_nodes], ei_l[1], 1)
    perm = _build_perm(deg_sum, n_pad)

    T = {}
    TT = {}
    per_core_maps = [dict() for _ in range(N_CORES)]
    for b, ei, ew in (("call", ei_c, inputs["edge_weight_call"]),
                      ("loc", ei_l, inputs["edge_weight_loc"])):
        Tb, TTb, pc = _prep_branch(ei, np.asarray(ew, F32), perm, n_nodes,
                                   n_pad)
        T[b] = Tb
        TT[b] = TTb
        for c in range(N_CORES):
            per_core_maps[c][f"{b}_gidx"] = pc[c]["gidx"]
            per_core_maps[c][f"{b}_S"] = pc[c]["S"]

    # features (permuted, padded, feature-major)
    x = np.nan_to_num(np.asarray(inputs["x"], F32))
    emb = np.asarray(inputs["emb"], F32)
    x_pad = np.zeros((n_pad, x.shape[1]), F32)
    emb_pad = np.zeros((n_pad, emb.shape[1]), F32)
    x_pad[perm[:n_nodes]] = x
    emb_pad[perm[:n_nodes]] = emb
    for c in range(N_CORES):
        sl = slice(c * NL, (c + 1) * NL)
        per_core_maps[c]["xT"] = np.ascontiguousarray(x_pad[sl].T)
        per_core_maps[c]["embT"] = np.ascontiguousarray(emb_pad[sl].T)

    # weights (shared)
    comb_W = np.asarray(inputs["comb_W"], F32)
    Wc1 = comb_W[:, :emb.shape[1]]
    Wc2 = comb_W[:, emb.shape[1]:]
    Wx = Wc2 @ np.asarray(inputs["ft_W"], F32)  # [H, IN] (ft_b == 0)
    common = {
        "WxT": np.ascontiguousarray(Wx.T),
        "Wc1T": np.ascontiguousarray(Wc1.T),
        "comb_b_rep": _rep(inputs["comb_b"]),
    }
    for b in _BRANCHES:
        Ws = np.asarray(inputs["call_W" if b == "call" else "loc_W"], F32)
        bs = np.asarray(inputs["call_b" if b == "call" else "loc_b"], F32)
        A, B = _affine_bn(np.asarray(inputs[f"bn_{b}"], F32))
        WT = np.concatenate([Ws[l].T for l in range(L)], axis=1)  # [H, L*H]
        common[f"{b}_WT"] = np.ascontiguousarray(WT)
        for l in range(L):
            common[f"{b}{l}_bias_rep"] = _rep(bs[l])
        common[f"{b}_A_rep"] = _rep(A)
        common[f"{b}_B_rep"] = _rep(B)
    fus_W = np.asarray(inputs["fus_W"], F32)
    Af, Bf = _affine_bn(np.asarray(inputs["bn_fus"], F32))
    common.update({
        "Wf1T": np.ascontiguousarray(fus_W[:, :H].T),
        "Wf2T": np.ascontiguousarray(fus_W[:, H:].T),
        "fus_b_rep": _rep(inputs["fus_b"]),
        "Af_rep": _rep(Af),
        "Bf_rep": _rep(Bf),
        "linW_rep": _rep(np.asarray(inputs["lin_W"], F32).reshape(H)),
    })
    lin_b = float(np.asarray(inputs["lin_b"], F32).reshape(-1)[0])

    meta = {
        "n_nodes": n_nodes,
        "n_pad": n_pad,
        "NL": NL,
        "NQ": n_pad // 4,
        "NBLK": NL // 128,
        "T": T,
        "TT": TT,
        "in_dim": x.shape[1],
        "emb_dim": emb.shape[1],
        "lin_b": lin_b,
    }
    for c in range(N_CORES):
        per_core_maps[c].update(common)
    return meta, per_core_maps, perm


# ----------------------------------------------------------------------------
# Device program
# ----------------------------------------------------------------------------

def _build_program(meta):
    import contextlib

    import concourse.bass as bass
    import concourse.bacc as bacc
    import concourse.mybir as mybir
    import concourse.tile as tile
    from concourse.masks import make_identity

    f32 = mybir.dt.float32
    bf = mybir.dt.bfloat16
    i16 = mybir.dt.int16
    AOT = mybir.AluOpType
    ACT = mybir.ActivationFunctionType

    NL = meta["NL"]
    NQ = meta["NQ"]
    NP = meta["n_pad"]
    NBLK = meta["NBLK"]
    NSB = (NBLK + 3) // 4
    IN_DIM = meta["in_dim"]
    EMB_DIM = meta["emb_dim"]
    T = meta["T"]
    TT = meta["TT"]

    nc = bacc.Bacc(None, num_devices=N_CORES, num_swdge_queues=4)

    inp = {}

    def ext(name, shape, dt=f32):
        inp[name] = nc.dram_tensor(name, list(shape), dt, kind="ExternalInput")
        return inp[name]

    ext("xT", [IN_DIM, NL])
    ext("embT", [EMB_DIM, NL])
    ext("WxT", [IN_DIM, H])
    ext("Wc1T", [EMB_DIM, H])
    ext("comb_b_rep", [128, H])
    for b in _BRANCHES:
        ext(f"{b}_WT", [H, L * H])
        for l in range(L):
            ext(f"{b}{l}_bias_rep", [128, H])
        ext(f"{b}_A_rep", [128, H])
        ext(f"{b}_B_rep", [128, H])
        ext(f"{b}_gidx", [128, TT[b] * 8], i16)
        ext(f"{b}_S", [128, TT[b] * 128], bf)
    ext("Wf1T", [H, H])
    ext("Wf2T", [H, H])
    ext("fus_b_rep", [128, H])
    ext("Af_rep", [128, H])
    ext("Bf_rep", [128, H])
    ext("linW_rep", [128, H])
    y_out = nc.dram_tensor("y", [128, NBLK], f32, kind="ExternalOutput")

    xw_bounce = [nc.dram_tensor(f"xw_bounce{i}", [NL, H], bf)
                 for i in range(2)]
    xw_full = [nc.dram_tensor(f"xw_full{i}", [NP, H], bf, addr_space="Shared")
               for i in range(2)]

    steps = [(b, l) for l in range(L) for b in _BRANCHES]
    steps = steps[:int(os.environ.get("GCN_STEPS", str(len(steps))))]

    # per-branch codegen schedule: list of (sb, q, blk_list, tile_gt_list)
    # and per (sb, bi): total tiles (for start/stop flags)
    sched = {}
    for b in _BRANCHES:
        Tb = T[b]
        calls = []
        sbbi_total = np.zeros((NSB, 4), np.int64)
        gt = 0
        for sb in range(NSB):
            blks = list(range(sb * 4, min(sb * 4 + 4, NBLK)))
            for q in range(4):
                for half in range(0, len(blks), 2):
                    tl = []
                    for blk in blks[half:half + 2]:
                        for _ in range(int(Tb[q][blk])):
                            tl.append((blk, gt))
                            gt += 1
                            sbbi_total[sb][blk - sb * 4] += 1
                    calls.append((sb, q, tl))
        assert gt == TT[b]
        sched[b] = (calls, sbbi_total)
    max_call = max(len(tl) for b in _BRANCHES
                   for (sb, q, tl) in sched[b][0])

    with tile.TileContext(nc) as tc:
        with contextlib.ExitStack() as ctx:
            konst = ctx.enter_context(tc.tile_pool(name="konst", bufs=1))
            hpool = ctx.enter_context(tc.tile_pool(name="hpool", bufs=1))
            mpool = ctx.enter_context(tc.tile_pool(name="mpool", bufs=1))
            cpool = ctx.enter_context(tc.tile_pool(name="cpool", bufs=4))
            spool = ctx.enter_context(tc.tile_pool(name="spool", bufs=4))
            upool = ctx.enter_context(tc.tile_pool(name="upool", bufs=2))
            xpool = ctx.enter_context(tc.tile_pool(name="xpool", bufs=2))
            pspool = ctx.enter_context(
                tc.tile_pool(name="ps", bufs=3, space="PSUM"))
            psa = ctx.enter_context(
                tc.tile_pool(name="psa", bufs=2, space="PSUM"))

            # ---- constants ----
            ident = konst.tile([128, 128], f32, tag="ident", name="ident")
            make_identity(nc, ident[:])
            iota_i = konst.tile([128, 128], mybir.dt.int32, tag="iota_i",
                                name="iota_i")
            nc.gpsimd.iota(iota_i[:], pattern=[[1, 128]], base=0,
                           channel_multiplier=0)
            iota_b = konst.tile([128, 128], bf, tag="iota_b", name="iota_b")
            nc.vector.tensor_copy(out=iota_b[:], in_=iota_i[:])

            def kload(name, shape, dt=f32):
                t = konst.tile(shape, dt, tag=name, name=name)
                nc.sync.dma_start(out=t[:], in_=inp[name][:, :])
                return t

            WT_sb = {b: kload(f"{b}_WT", [H, L * H]) for b in _BRANCHES}
            bias_rep = {(b, l): kload(f"{b}{l}_bias_rep", [128, H])
                        for b in _BRANCHES for l in range(L)}
            A_rep = {b: kload(f"{b}_A_rep", [128, H]) for b in _BRANCHES}
            B_rep = {b: kload(f"{b}_B_rep", [128, H]) for b in _BRANCHES}
            WxT_sb = kload("WxT", [IN_DIM, H])
            Wc1T_sb = kload("Wc1T", [EMB_DIM, H])
            comb_b_rep = kload("comb_b_rep", [128, H])
            Wf1T_sb = kload("Wf1T", [H, H])
            Wf2T_sb = kload("Wf2T", [H, H])
            fus_b_rep = kload("fus_b_rep", [128, H])
            Af_rep = kload("Af_rep", [128, H])
            Bf_rep = kload("Bf_rep", [128, H])
            linW_rep = kload("linW_rep", [128, H])

            TTmax = max(TT[b] for b in _BRANCHES)

            h_sb = {b: hpool.tile([128, NBLK, 128], f32, tag=f"h_{b}",
                                  name=f"h_{b}") for b in _BRANCHES}

            # ---- front: h0 = relu(emb@Wc1^T + x@Wx^T + comb_b), node-major --
            with tc.tile_pool(name="front", bufs=1) as fpool:
                for c0 in range(0, NBLK, 4):
                    nt = min(4, NBLK - c0)
                    n0 = c0 * 128
                    cw = nt * 128
                    xT_sb = fpool.tile([IN_DIM, 4 * 128], f32, tag="xT",
                                       name="xT")
                    embT_sb = fpool.tile([EMB_DIM, 4 * 128], f32, tag="embT",
                                         name="embT")
                    nc.sync.dma_start(out=xT_sb[:, :cw],
                                      in_=inp["xT"][:, n0:n0 + cw])
                    nc.sync.dma_start(out=embT_sb[:, :cw],
                                      in_=inp["embT"][:, n0:n0 + cw])
                    ps = psa.tile([128, 4, 128], f32, tag="pst", name="pst")
                    for t in range(nt):
                        nc.tensor.matmul(out=ps[:, t, :],
                                         lhsT=xT_sb[:, t * 128:(t + 1) * 128],
                                         rhs=WxT_sb[:],
                                         start=True, stop=False)
                        nc.tensor.matmul(out=ps[:, t, :],
                                         lhsT=embT_sb[:, t * 128:(t + 1) * 128],
                                         rhs=Wc1T_sb[:],
                                         start=False, stop=True)
                    tmp = upool.tile([128, 4, 128], f32, tag="ft",
                                     name="ft")
                    nc.vector.tensor_tensor(
                        out=tmp[:, :nt, :], in0=ps[:, :nt, :],
                        in1=comb_b_rep[:, None, :].to_broadcast(
                            [128, nt, 128]),
                        op=AOT.add)
                    nc.vector.tensor_scalar(
                        out=h_sb["call"][:, c0:c0 + nt, :], in0=tmp[:, :nt, :],
                        scalar1=0.0, scalar2=None, op0=AOT.max)
                    nc.vector.tensor_copy(out=h_sb["loc"][:, c0:c0 + nt, :],
                                          in_=h_sb["call"][:, c0:c0 + nt, :])

            # ---- phases ----
            def phase_a(k):
                b, l = steps[k]
                pp = k % 2
                for c0 in range(0, NBLK, 4):
                    nt = min(4, NBLK - c0)
                    cw = nt * 128
                    pst = psa.tile([128, 4, 128], f32, tag="pst", name="pst")
                    for t in range(nt):
                        nc.tensor.matmul(out=pst[:, t, :],
                                         lhsT=h_sb[b][:, c0 + t, :],
                                         rhs=ident[:], is_transpose=True,
                                         start=True, stop=True)
                    hT = xpool.tile([128, 4, 128], f32, tag="hT", name="hT")
                    nc.scalar.activation(out=hT[:, :nt, :], in_=pst[:, :nt, :],
                                         func=ACT.Copy, bias=0.0, scale=1.0)
                    ps2 = psa.tile([128, 4, 128], f32, tag="ps2", name="ps2")
                    for t in range(nt):
                        nc.tensor.matmul(out=ps2[:, t, :],
                                         lhsT=hT[:, t, :],
                                         rhs=WT_sb[b][:, l * H:(l + 1) * H],
                                         start=True, stop=True)
                    stg = xpool.tile([128, 4, 128], bf, tag="stg", name="stg")
                    nc.vector.tensor_copy(out=stg[:, :nt, :],
                                          in_=ps2[:, :nt, :])
                    nc.sync.dma_start(
                        out=xw_bounce[pp][c0 * 128:c0 * 128 + cw, :].rearrange(
                            "(t p) h -> p t h", p=128),
                        in_=stg[:, :nt, :])
                nc.gpsimd.collective_compute(
                    "AllGather", AOT.bypass,
                    replica_groups=[list(range(N_CORES))],
                    ins=[xw_bounce[pp][:, :].opt()],
                    outs=[xw_full[pp][:, :].opt()],
                )

            def phase_b(k):
                b, l = steps[k]
                pp = k % 2
                gidx_t = mpool.tile([128, TTmax * 8], i16, tag="gidx",
                                    name="gidx")
                nc.sync.dma_start(out=gidx_t[:, :TT[b] * 8],
                                  in_=inp[f"{b}_gidx"][:, :])
                calls, sbbi_total = sched[b]
                calls_by_sb = {}
                for (sb, q, tl) in calls:
                    calls_by_sb.setdefault(sb, []).append((q, tl))
                qcnt = 0
                for sb in range(NSB):
                    nb = min(4, NBLK - sb * 4)
                    ps_bi = [pspool.tile([128, 512], f32, tag=f"psb{i}",
                                         name=f"psb{i}", bufs=1)
                             for i in range(nb)]
                    seen = [0, 0, 0, 0]
                    for (q, tl) in calls_by_sb[sb]:
                        if not tl:
                            continue
                        ntile = len(tl)
                        gt0 = tl[0][1]
                        C = cpool.tile([128, max_call, 128], bf, tag="C",
                                       name="C")
                        nc.gpsimd.dma_gather(
                            C[:, :ntile, :],
                            xw_full[pp][q * NQ:(q + 1) * NQ, :],
                            gidx_t[:, gt0 * 8:(gt0 + ntile) * 8],
                            ntile * 128, ntile * 128, H,
                            single_packet=False, queue_num=qcnt % 4)
                        qcnt += 1
                        Sg = spool.tile([128, max_call, 128], bf, tag="Sg",
                                        name="Sg")
                        nc.sync.dma_start(
                            out=Sg[:, :ntile, :],
                            in_=inp[f"{b}_S"][:, gt0 * 128:
                                              (gt0 + ntile) * 128].rearrange(
                                "p (t h) -> p t h", h=128))
                        for i, (blk, gt) in enumerate(tl):
                            bi = blk - sb * 4
                            first = seen[bi] == 0
                            last = seen[bi] == sbbi_total[sb][bi] - 1
                            nc.tensor.matmul(out=ps_bi[bi][:, 0:128],
                                             lhsT=Sg[:, i, :], rhs=C[:, i, :],
                                             start=bool(first),
                                             stop=bool(last))
                            seen[bi] += 1
                    _update(b, l, sb, ps_bi, nb)

            def _update(b, l, sb, ps_bi, nb):
                for bi in range(nb):
                    t1 = upool.tile([128, 128], f32, tag="t1", name="t1")
                    nc.vector.tensor_tensor(
                        out=t1[:], in0=ps_bi[bi][:, 0:128],
                        in1=bias_rep[(b, l)][:], op=AOT.add)
                    t2 = upool.tile([128, 128], f32, tag="t2", name="t2")
                    nc.scalar.activation(out=t2[:], in_=t1[:],
                                         func=ACT.Relu, bias=0.0, scale=1.0)
                    nc.vector.tensor_tensor(out=t2[:], in0=t2[:],
                                            in1=A_rep[b][:], op=AOT.mult)
                    nc.vector.tensor_tensor(out=t2[:], in0=t2[:],
                                            in1=B_rep[b][:], op=AOT.add)
                    nc.vector.tensor_tensor(
                        out=h_sb[b][:, sb * 4 + bi, :],
                        in0=h_sb[b][:, sb * 4 + bi, :],
                        in1=t2[:], op=AOT.add)

            for k in range(min(2, len(steps))):
                phase_a(k)
            for k in range(2, len(steps)):
                phase_b(k - 2)
                phase_a(k)
            for k in range(max(0, len(steps) - 2), len(steps)):
                phase_b(k)

            # ---- back: fuse + BN + head (node-major) ----
            lin_b = meta["lin_b"]
            for c0 in range(0, NBLK, 4):
                nt = min(4, NBLK - c0)
                hTs = {}
                for bb in _BRANCHES:
                    pst = psa.tile([128, 4, 128], f32, tag="pst", name="pst")
                    for t in range(nt):
                        nc.tensor.matmul(out=pst[:, t, :],
                                         lhsT=h_sb[bb][:, c0 + t, :],
                                         rhs=ident[:], is_transpose=True,
                                         start=True, stop=True)
                    hT = xpool.tile([128, 4, 128], f32,
                                    tag=("hT" if bb == "call" else "hT_loc"),
                                    name="hTb")
                    nc.scalar.activation(out=hT[:, :nt, :], in_=pst[:, :nt, :],
                                         func=ACT.Copy, bias=0.0, scale=1.0)
                    hTs[bb] = hT
                ps = psa.tile([128, 4, 128], f32, tag="ps2", name="ps2")
                for t in range(nt):
                    nc.tensor.matmul(out=ps[:, t, :], lhsT=hTs["call"][:, t, :],
                                     rhs=Wf1T_sb[:], start=True, stop=False)
                    nc.tensor.matmul(out=ps[:, t, :], lhsT=hTs["loc"][:, t, :],
                                     rhs=Wf2T_sb[:], start=False, stop=True)
                hf = upool.tile([128, 4, 128], f32, tag="ft", name="ft")
                nc.vector.tensor_tensor(
                    out=hf[:, :nt, :], in0=ps[:, :nt, :],
                    in1=fus_b_rep[:, None, :].to_broadcast([128, nt, 128]),
                    op=AOT.add)
                nc.vector.tensor_scalar(out=hf[:, :nt, :], in0=hf[:, :nt, :],
                                        scalar1=0.0, scalar2=None, op0=AOT.max)
                nc.vector.tensor_tensor(
                    out=hf[:, :nt, :], in0=hf[:, :nt, :],
                    in1=Af_rep[:, None, :].to_broadcast([128, nt, 128]),
                    op=AOT.mult)
                nc.vector.tensor_tensor(
                    out=hf[:, :nt, :], in0=hf[:, :nt, :],
                    in1=Bf_rep[:, None, :].to_broadcast([128, nt, 128]),
                    op=AOT.add)
                nc.vector.tensor_tensor(
                    out=hf[:, :nt, :], in0=hf[:, :nt, :],
                    in1=linW_rep[:, None, :].to_broadcast([128, nt, 128]),
                    op=AOT.mult)
                yred = upool.tile([128, 4], f32, tag="yred", name="yred")
                import concourse.mybir as _mb
                nc.vector.tensor_reduce(
                    out=yred[:, :nt], in_=hf[:, :nt, :],
                    axis=_mb.AxisListType.X, op=AOT.add)
                ycl = upool.tile([128, 4], f32, tag="ycl", name="ycl")
                nc.vector.tensor_scalar(out=ycl[:, :nt], in0=yred[:, :nt],
                                        scalar1=lin_b, scalar2=-10.0,
                                        op0=AOT.add, op1=AOT.max)
                nc.vector.tensor_scalar(out=ycl[:, :nt], in0=ycl[:, :nt],
                                        scalar1=10.0, scalar2=None,
                                        op0=AOT.min)
                nc.sync.dma_start(out=y_out[:, c0:c0 + nt], in_=ycl[:, :nt])

            if os.environ.get("GCN_DEBUG_H") == "1":
                for b in _BRANCHES:
                    dbg = nc.dram_tensor(f"dbg_h_{b}", [128, NBLK * 128], f32,
                                         kind="ExternalOutput")
                    nc.sync.dma_start(
                        out=dbg[:, :],
                        in_=h_sb[b][:].rearrange("p t h -> p (t h)"))

    nc.compile()
    return nc


# ----------------------------------------------------------------------------
# Entry point
# ----------------------------------------------------------------------------

def kernel(**inputs) -> np.ndarray:
    from concourse.bass_utils import run_bass_kernel_spmd

    meta, per_core_maps, perm = _host_prep(inputs)
    nc = _build_program(meta)
    trace = os.environ.get("GCN_TRACE", "") == "1"
    kw = {}
    if trace:
        kw = dict(trace=True)
    res = run_bass_kernel_spmd(nc, per_core_maps,
                               core_ids=list(range(N_CORES)), **kw)
    if trace:
        kernel.last_exec_time_ns = res.exec_time_ns
        kernel.last_trace = (res.instructions_and_trace[1]
                             if res.instructions_and_trace else None)
    kernel.last_results = res.results
    n_nodes = meta["n_nodes"]
    NBLK = meta["NBLK"]
    # y[p, t] = node t*128 + p (per core)
    y_pad = np.concatenate(
        [np.asarray(res.results[c]["y"]).T.reshape(-1) for c in range(N_CORES)])
    out = y_pad[perm[:n_nodes]].astype(np.float32).reshape(n_nodes, 1)
    return out
